# revision 4
# baseline (speedup 1.0000x reference)
"""Trainium2 Bass kernel for nn_C4ByteTransformer (4-step carry-propagation
softmax table lookup).

Contract: kernel(**inputs) takes FULL inputs (a_emb[4,256], b_emb[4,256],
W1[514,131072], W2_sum[131072,256], W2_carry[131072,2]) and returns the full
[4,256] float32 output.

Math: the tables are the canonical byte-add lookup structure (verified
exactly on host, with a numpy fallback otherwise):
  scores_i[k] = a_emb[i, a] + b_emb[i, b] + carry[c],  k = 512a + 2b + c
  weights = softmax(10*(scores - 2.5));  out_i = weights @ W2_sum;
  carry' = weights @ W2_carry,  W2_sum[k, (a+b+c) & 255] = 1,
  W2_carry[k, a+b+c >= 256] = 1.
Because exp is multiplicative over the separable score, with
EA[a] = exp(10 a_emb[i,a]), EB[b] = exp(10 b_emb[i,b]) and
s = sigmoid(20 carry_1 - 10) (= F1/(F0+F1)):
  out_i[m] = ((1-s) cyc[m] + s cyc[(m-1) mod 256]) / (ZA ZB)
  carry'_1 = (U + V s) / (ZA ZB)
where cyc = 256-point cyclic convolution of EA and EB,
U = sum_{a+b>=256} EA[a]EB[b], V = sum_{a+b=255} EA[a]EB[b].
The 131072-entry table never has to be touched. The host pre-replicates
b_emb into Hankel-window layout (pure input packing); the device exps it
once and runs eight float32r matmuls whose lhsT is a step-masked exp(a)
block (off-step columns are exp(-50) ~ 0), so all eight accumulate into
one [4, 256] PSUM tile, landing step-on-partition with no transpose and
no DRAM round trips. U comes from suffix sums of EB via one constant
triangular matmul; the sums are broadcast onto partitions 0-3 by the
reduction matmul itself, so the 3-op-per-step carry chain and the final
combine run without any cross-partition scatter. One NeuronCore, ~1.1 MB
of input DMA, no collectives, no intermediate DRAM traffic.
"""

import os

import numpy as np

NSTEP = 4
D = 256
NE = 131072

_CACHE = {}

LAST_EXEC_TIME_NS = None
LAST_RESULT = None


def _build_nc():
    import concourse.bacc as bacc
    import concourse.mybir as mybir
    import concourse.tile as tile

    f32 = mybir.dt.float32
    f32r = mybir.dt.float32r
    mult = mybir.AluOpType.mult
    add = mybir.AluOpType.add
    subtract = mybir.AluOpType.subtract
    Exp = mybir.ActivationFunctionType.Exp

    nc = bacc.Bacc("TRN2", target_bir_lowering=False, debug=False,
                   num_devices=1)

    # Inputs (host pre-packed; see _prep_inputs).
    # pk packs [b8 | a8 | a8m] = 8 + 8 + 32 f32 per partition.
    bwin = nc.dram_tensor("bwin", [128, NSTEP, 256], f32,
                          kind="ExternalInput")
    pk = nc.dram_tensor("pk", [128, 48], f32, kind="ExternalInput")
    tri = nc.dram_tensor("tri", [128, 128], f32r, kind="ExternalInput")
    msk = nc.dram_tensor("msk", [NSTEP, 8], f32, kind="ExternalInput")
    out = nc.dram_tensor("out", [NSTEP, D], f32, kind="ExternalOutput")

    S0 = float(1.0 / (1.0 + np.exp(10.0)))  # sigmoid(-10): step-0 carry wt

    with tile.TileContext(nc) as tc:
        with (
            tc.tile_pool(name="sb", bufs=1) as sb,
            tc.tile_pool(name="small", bufs=1) as small,
            tc.tile_pool(name="psA", bufs=1, space="PSUM") as psA,
            tc.tile_pool(name="psC", bufs=1, space="PSUM") as psC,
            tc.tile_pool(name="psD", bufs=1, space="PSUM") as psD,
        ):
            bias0_128 = small.tile([128, 1], f32)
            nc.vector.memset(bias0_128[:], 0.0)
            bias10_4 = small.tile([NSTEP, 1], f32)
            nc.vector.memset(bias10_4[:], 10.0)
            one_sb = sb.tile([128, NSTEP], f32)
            nc.vector.memset(one_sb[:], 1.0)
            one_4 = small.tile([NSTEP, 1], f32)
            nc.vector.memset(one_4[:], 1.0)
            s0_4 = small.tile([NSTEP, 1], f32)
            nc.vector.memset(s0_4[:], S0)

            # Inputs. Bulk on the gpsimd queue (the sync queue moves bulk
            # ~4x slower); tri leads the scalar queue, which also primes the
            # activation-table load before the exp stream begins.
            tri_sb = sb.tile([128, 128], f32r)
            nc.scalar.dma_start(tri_sb[:], tri[:])
            msk_sb = small.tile([NSTEP, 8], f32)
            nc.scalar.dma_start(msk_sb[:], msk[:])
            pk_sb = sb.tile([128, 48], f32)
            nc.sync.dma_start(pk_sb[:], pk[:])
            bwin_sb = sb.tile([128, NSTEP, 256], f32)
            for i in range(NSTEP):
                nc.gpsimd.dma_start(bwin_sb[:, i:i + 1, :], bwin[:, i:i + 1, :])

            # Fused exps for [b8 | a8 | a8m]; f32r rounding everywhere is
            # ~1e-5 relative, well inside tolerance. f32-typed views below.
            # b8 part first: it gates the suffix-sum chain.
            epk = sb.tile([128, 48], f32r)
            nc.scalar.activation(epk[:, 0:8], pk_sb[:, 0:8], Exp,
                                 bias=bias0_128[:], scale=10.0)
            nc.scalar.activation(epk[:, 8:48], pk_sb[:, 8:48], Exp,
                                 bias=bias0_128[:], scale=10.0)

            def ebv(bh):  # exp(b8)[:, bh, :] as f32
                return epk[:, 4 * bh:4 * bh + 4].bitcast(f32)

            def eav(ah):  # exp(a8)[:, ah, :] as f32
                return epk[:, 8 + 4 * ah:12 + 4 * ah].bitcast(f32)

            def eamv(ah, i):  # masked exp(a8) lhsT block, f32r
                o = 16 + 16 * ah + 4 * i
                return epk[:, o:o + 4]

            # Hankel windows of exp(b) (one cyclic period; the conv matmuls
            # wrap around it), per step so they start early.
            ewin = sb.tile([128, NSTEP, 256], f32r)
            ew_acts = []
            for i in range(NSTEP):
                ew_acts.append((ewin[:, i:i + 1, :], bwin_sb[:, i:i + 1, :]))
            ew_insts = {}
            for i in range(2):
                ew_insts[i] = nc.scalar.activation(
                    ew_acts[i][0], ew_acts[i][1], Exp, bias=bias0_128[:],
                    scale=10.0)

            # ---- Partial suffix sums (within-half only, float32r):
            # suf[p, tc, i] = sum_{q > p} EB_i[128 tc + q]. The cross-half
            # ZB1 contribution to U is restored as + ZA1 ZB1 after reduction.
            suf_ps = psA.tile([128, 2, NSTEP], f32)
            nc.tensor.matmul(suf_ps[:, 0, :], lhsT=tri_sb[:],
                             rhs=epk[:, 0:4], start=True, stop=True)
            nc.tensor.matmul(suf_ps[:, 1, :], lhsT=tri_sb[:],
                             rhs=epk[:, 4:8], start=True, stop=True)
            suf_sb = sb.tile([128, 2, NSTEP], f32)
            nc.vector.tensor_copy(out=suf_sb[:], in_=suf_ps[:])

            # ---- U/V element products; partition-reduce via ones matmul.
            # lhsT = 4 identical ones columns -> sums broadcast to parts 0-3.
            scr = sb.tile([128, 4, NSTEP], f32)
            nc.vector.tensor_tensor(out=scr[:, 0, :], in0=eav(0),
                                    in1=suf_sb[:, 1, :], op=mult)
            nc.vector.tensor_tensor(out=scr[:, 1, :], in0=eav(1),
                                    in1=suf_sb[:, 0, :], op=mult)
            nc.vector.tensor_tensor(out=scr[:, 2, :], in0=eav(0),
                                    in1=ebv(1), op=mult)
            nc.vector.tensor_tensor(out=scr[:, 3, :], in0=eav(1),
                                    in1=ebv(0), op=mult)

            red_ps = psC.tile([NSTEP, 8, NSTEP], f32)
            ones4 = one_sb[:, 0:4]
            nc.tensor.matmul(red_ps[:, 0:4, :].opt(), lhsT=ones4,
                             rhs=scr[:].opt(), start=True, stop=True)
            nc.tensor.matmul(red_ps[:, 4:8, :].opt(), lhsT=ones4,
                             rhs=epk[:, 0:16].bitcast(f32), start=True,
                             stop=True)
            red_sb = small.tile([NSTEP, 8, NSTEP], f32)
            nc.vector.tensor_copy(out=red_sb[:], in_=red_ps[:])

            # sums[p, k, i]: k = 0:U, 1:V, 2:ZB, 3:ZA (fold the ah pairs)
            sums = small.tile([NSTEP, 4, NSTEP], f32)
            for k in range(4):
                nc.vector.tensor_tensor(
                    out=sums[:, k:k + 1, :],
                    in0=red_sb[:, 2 * k, :].unsqueeze(1),
                    in1=red_sb[:, 2 * k + 1, :].unsqueeze(1), op=add)
            # Cross-half correction: U += ZA1 ZB1 (see suffix-sum note).
            tzz = small.tile([NSTEP, NSTEP], f32)
            nc.vector.tensor_tensor(out=tzz[:], in0=red_sb[:, 5, :],
                                    in1=red_sb[:, 7, :], op=mult)
            nc.vector.tensor_tensor(out=sums[:, 0, :], in0=sums[:, 0, :],
                                    in1=tzz[:], op=add)
            zab = small.tile([NSTEP, NSTEP], f32)
            nc.vector.tensor_tensor(out=zab[:], in0=sums[:, 2, :],
                                    in1=sums[:, 3, :], op=mult)
            zbi = small.tile([NSTEP, NSTEP], f32)
            nc.vector.reciprocal(zbi[:], zab[:])
            # Pre-divided U/V so the carry chain is ACT->add->recip->stt.
            uz = small.tile([NSTEP, NSTEP], f32)
            nc.vector.tensor_tensor(out=uz[:], in0=sums[:, 0, :],
                                    in1=zbi[:], op=mult)
            vz = small.tile([NSTEP, NSTEP], f32)
            nc.vector.tensor_tensor(out=vz[:], in0=sums[:, 1, :],
                                    in1=zbi[:], op=mult)

            # ---- Carry chain on partitions 0-3 ----
            # s_i = F1/(F0+F1) = 1/(1 + exp(10 - 20 c1)); step 0 is the
            # constant sigmoid(-10). Exp-only so the ACT table never swaps;
            # the last window exp is interleaved after step 1's ACT.
            cc = small.tile([NSTEP, 1], f32)
            rr = small.tile([NSTEP, 1], f32)
            ss = small.tile([NSTEP, 1], f32)
            tt = small.tile([NSTEP, 1], f32)
            scal = small.tile([NSTEP, 8], f32)  # (beta_i, alpha_i) pairs
            lsb = small.tile([NSTEP, 2], f32)  # row i: (beta_i, alpha_i)
            for i in range(NSTEP):
                if i == 0:
                    ss_i = s0_4[:]
                else:
                    # r = exp(10 - 20 c1); s = 1/(1 + r). The remaining
                    # window exps are interleaved between the chain's ACTs;
                    # the explicit dep pins the interleave (the scheduler
                    # would otherwise hoist all window exps first).
                    rec_act = nc.scalar.activation(rr[:], cc[:], Exp,
                                                   bias=bias10_4[:],
                                                   scale=-20.0)
                    if i + 1 < NSTEP:
                        ew_insts[i + 1] = nc.scalar.activation(
                            ew_acts[i + 1][0], ew_acts[i + 1][1], Exp,
                            bias=bias0_128[:], scale=10.0)
                        tile.add_dep_helper(ew_insts[i + 1].ins, rec_act.ins,
                                            False, "carry ACT before window")
                    nc.vector.tensor_tensor(out=tt[:], in0=rr[:],
                                            in1=one_4[:], op=add)
                    nc.vector.reciprocal(ss[:], tt[:])
                    ss_i = ss[:]
                if i + 1 < NSTEP:
                    # c1' = (V s + U) / ZAB = Vz s + Uz
                    nc.vector.scalar_tensor_tensor(
                        out=cc[:], in0=vz[:, i:i + 1], scalar=ss_i,
                        in1=uz[:, i:i + 1], op0=mult, op1=add)
                beta = scal[:, 2 * i:2 * i + 1]
                nc.vector.tensor_tensor(out=beta, in0=ss_i,
                                        in1=zbi[:, i:i + 1], op=mult)
                nc.vector.tensor_tensor(out=scal[:, 2 * i + 1:2 * i + 2],
                                        in0=zbi[:, i:i + 1], in1=beta,
                                        op=subtract)

            # lsb[i, 0:2] = (beta_i, alpha_i) selected via the one-hot mask.
            tmp8 = small.tile([NSTEP, 8], f32)
            nc.vector.tensor_tensor(out=tmp8[:], in0=scal[:], in1=msk_sb[:],
                                    op=mult)
            nc.vector.tensor_tensor(out=lsb[:], in0=tmp8[:, 0:2],
                                    in1=tmp8[:, 2:4], op=add)
            nc.vector.tensor_tensor(out=tmp8[:, 4:6], in0=tmp8[:, 4:6],
                                    in1=tmp8[:, 6:8], op=add)
            nc.vector.tensor_tensor(out=lsb[:], in0=lsb[:],
                                    in1=tmp8[:, 4:6], op=add)

            # ---- Convolutions: 12 matmuls accumulate into prt[i, m] ----
            # lhsT = masked exp(a) block (off-step columns exp(-50) ~ 0);
            # rhs is one cyclic period of the window: ah=0 uses it straight,
            # ah=1 rotated by 128 (two wrapped halves). float32r PE mode.
            prt = psD.tile([NSTEP, 256], f32)
            for i in range(NSTEP):
                nc.tensor.matmul(prt[:], lhsT=eamv(0, i), rhs=ewin[:, i, :],
                                 start=(i == 0), stop=False)
                nc.tensor.matmul(prt[:, 0:128], lhsT=eamv(1, i),
                                 rhs=ewin[:, i, 128:256], start=False,
                                 stop=False)
                nc.tensor.matmul(prt[:, 128:256], lhsT=eamv(1, i),
                                 rhs=ewin[:, i, 0:128], start=False,
                                 stop=(i == NSTEP - 1))

            # out[i, m] = alpha_i cyc[m] + beta_i cyc[m-1], straight off PSUM.
            comb = small.tile([NSTEP, D], f32)
            nc.vector.tensor_scalar(out=comb[:], in0=prt[:],
                                    scalar1=lsb[:, 1:2], scalar2=None,
                                    op0=mult)
            nc.vector.scalar_tensor_tensor(out=comb[:, 1:256],
                                           in0=prt[:, 0:255],
                                           scalar=lsb[:, 0:1],
                                           in1=comb[:, 1:256],
                                           op0=mult, op1=add)
            nc.vector.scalar_tensor_tensor(out=comb[:, 0:1],
                                           in0=prt[:, 255:256],
                                           scalar=lsb[:, 0:1],
                                           in1=comb[:, 0:1],
                                           op0=mult, op1=add)
            nc.sync.dma_start(out[:], comb[:])

    nc.compile()
    return nc


def _structure_ok(W1, W2_sum, W2_carry):
    """Exact check that the tables are the canonical byte-add structure."""
    k = np.arange(NE)
    a = k >> 9
    b = (k >> 1) & 255
    c = k & 1
    total = a + b + c
    if W1.shape != (514, NE) or W2_sum.shape != (NE, D):
        return False
    if W2_carry.shape != (NE, 2):
        return False
    if not (W1[a, k] == 1.0).all():
        return False
    if not (W1[256 + b, k] == 1.0).all():
        return False
    if not (W1[512 + c, k] == 1.0).all():
        return False
    if np.abs(W1).sum(dtype=np.float64) != 3.0 * NE:
        return False
    if not (W2_sum[k, total & 255] == 1.0).all():
        return False
    if np.abs(W2_sum).sum(dtype=np.float64) != float(NE):
        return False
    if not (W2_carry[k, (total >= 256).astype(np.int64)] == 1.0).all():
        return False
    if np.abs(W2_carry).sum(dtype=np.float64) != float(NE):
        return False
    return True


def _numpy_fallback(a_emb, b_emb, W1, W2_sum, W2_carry):
    carry = np.zeros(2, dtype=np.float64)
    carry[0] = 1.0
    outs = []
    W1 = W1.astype(np.float64)
    for i in range(NSTEP):
        x = np.concatenate([a_emb[i], b_emb[i], carry]).astype(np.float64)
        scores = x @ W1
        z = (scores - 2.5) * 10.0
        z -= z.max()
        w = np.exp(z)
        w /= w.sum()
        outs.append(w @ W2_sum.astype(np.float64))
        carry = w @ W2_carry.astype(np.float64)
    return np.stack(outs).astype(np.float32)


def _prep_inputs(a_emb, b_emb):
    p = np.arange(128)
    # bwin[j, i, x] = b_emb[i, (j + x + 129) mod 256], one cyclic period
    b_ext = np.take(b_emb, (np.arange(383) + 129) % 256, axis=1)
    bwin = np.ascontiguousarray(
        np.lib.stride_tricks.sliding_window_view(b_ext, 256, axis=1)
        .transpose(1, 0, 2)
    ).astype(np.float32)
    # a8[p, ah, i] = a_emb[i, 128 ah + 127 - p]
    a_r = a_emb[:, ::-1]
    a8 = np.ascontiguousarray(
        a_r.reshape(NSTEP, 2, 128)[:, ::-1, :].transpose(2, 1, 0)
    ).astype(np.float32)
    # a8m: step-masked copy (off-step columns -5 -> exp(10x) ~ 2e-22)
    a8m = np.full((128, 2, NSTEP, NSTEP), -5.0, dtype=np.float32)
    for i in range(NSTEP):
        a8m[:, :, i, i] = a8[:, :, i]
    # b8[p, bh, i] = b_emb[i, 128 bh + p]
    b8 = np.ascontiguousarray(
        b_emb.reshape(NSTEP, 2, 128).transpose(2, 1, 0)
    ).astype(np.float32)
    pk = np.concatenate(
        [b8.reshape(128, 8), a8.reshape(128, 8), a8m.reshape(128, 32)],
        axis=1,
    )
    tri = (p[:, None] >= p[None, :] + 1).astype(np.float32)
    msk = (np.arange(8)[None, :] // 2 == np.arange(NSTEP)[:, None]).astype(
        np.float32
    )
    return {"bwin": bwin, "pk": pk, "tri": tri, "msk": msk}


def kernel(a_emb, b_emb, W1, W2_sum, W2_carry):
    global LAST_EXEC_TIME_NS, LAST_RESULT
    a_emb = np.asarray(a_emb, dtype=np.float32)
    b_emb = np.asarray(b_emb, dtype=np.float32)
    W1 = np.asarray(W1, dtype=np.float32)
    W2_sum = np.asarray(W2_sum, dtype=np.float32)
    W2_carry = np.asarray(W2_carry, dtype=np.float32)

    if not _structure_ok(W1, W2_sum, W2_carry):
        return _numpy_fallback(a_emb, b_emb, W1, W2_sum, W2_carry)

    from concourse.bass_utils import run_bass_kernel_spmd

    if "nc" not in _CACHE:
        _CACHE["nc"] = _build_nc()
    nc = _CACHE["nc"]

    in_map = _prep_inputs(a_emb, b_emb)
    trace = os.environ.get("KERNEL_TRACE", "") == "1"
    res = run_bass_kernel_spmd(nc, [in_map], [0], trace=trace)
    LAST_EXEC_TIME_NS = res.exec_time_ns
    LAST_RESULT = res
    return np.asarray(res.results[0]["out"], dtype=np.float32)



# revision 20
# speedup vs baseline: 1.0852x; 1.0852x over previous
"""Trainium2 Bass kernel for nn_C4ByteTransformer (4-step carry-propagation
softmax table lookup).

Contract: kernel(**inputs) takes FULL inputs (a_emb[4,256], b_emb[4,256],
W1[514,131072], W2_sum[131072,256], W2_carry[131072,2]) and returns the full
[4,256] float32 output.

Math: the tables are the canonical byte-add lookup structure (verified
exactly on host, with a numpy fallback otherwise):
  scores_i[k] = a_emb[i, a] + b_emb[i, b] + carry[c],  k = 512a + 2b + c
  weights = softmax(10*(scores - 2.5));  out_i = weights @ W2_sum;
  carry' = weights @ W2_carry,  W2_sum[k, (a+b+c) & 255] = 1,
  W2_carry[k, a+b+c >= 256] = 1.
Because exp is multiplicative over the separable score, with
EA[a] = exp(10 a_emb[i,a]), EB[b] = exp(10 b_emb[i,b]) and
s = sigmoid(20 carry_1 - 10) (= F1/(F0+F1)):
  out_i[m] = ((1-s) cyc[m] + s cyc[(m-1) mod 256]) / (ZA ZB)
  carry'_1 = (U + V s) / (ZA ZB)
where cyc = 256-point cyclic convolution of EA and EB,
U = sum_{a+b>=256} EA[a]EB[b], V = sum_{a+b=255} EA[a]EB[b].
The 131072-entry table never has to be touched.

V2 latency structure (the kernel is launch-overhead dominated; ~13.1us of
the exec time is fixed preamble/DMA-latency/teardown measured with a
trivial kernel):
 - The carry recursion is rewritten in tanh form:
     t_{i+1} = tanh(5 vz_i t_i + (10 uz_i + 5 vz_i - 5)),  s = (1+t)/2,
   which the ACT engine evaluates as ONE Tanh per step with per-partition
   scale/bias APs. Tanh lives in the same activation-function set as Exp
   (exp_and_others), so no table swap and no DVE round-trips: the whole
   chain is 3 back-to-back ACT ops.
 - U/V/Z sums: one [128x128] triangular matmul (tri rides the pk DMA as
   f32, bitcast to f32r) gives within-half suffix sums for both halves at
   once; element products and folds are split across DVE and GpSimd; one
   ones-lhsT matmul pair reduces partitions and broadcasts to partitions
   0-3.
 - Final combine: out = zsel*(cyc + ssel*(rot(cyc)-cyc)). d = rot-cyc and
   the zsel pre-scales run on DVE while the chain finishes, leaving a
   single [4,256] op after the step-select.
 - DMA: pkA (small, gates everything) on the sync queue, pkB (a8m+tri) on
   the tensor queue, the four Hankel windows split over gpsimd+vector
   queues. Constants (ones, diag mask) ride pkA; one activation-table
   load; 5 input DMA instructions total.
"""

import os

import numpy as np

NSTEP = 4
D = 256
NE = 131072

_CACHE = {}

LAST_EXEC_TIME_NS = None
LAST_RESULT = None

T0 = float(np.tanh(-5.0))  # chain state for step 0 (s0 = sigmoid(-10))


def _build_nc():
    import concourse.bacc as bacc
    import concourse.mybir as mybir
    import concourse.tile as tile

    f32 = mybir.dt.float32
    f32r = mybir.dt.float32r
    mult = mybir.AluOpType.mult
    add = mybir.AluOpType.add
    subtract = mybir.AluOpType.subtract
    Exp = mybir.ActivationFunctionType.Exp
    Tanh = mybir.ActivationFunctionType.Tanh

    nc = bacc.Bacc("TRN2", target_bir_lowering=False, debug=False,
                   num_devices=1)

    # pkA [128, 28]: b8 (0:8, [bh,i]), a8 (8:16, [ah,i]), ones4 (16:20),
    # mskZ = diag(1.0) rows 0-3 (20:24), mskT = diag(0.5) (24:28).
    pka = nc.dram_tensor("pka", [128, 28], f32, kind="ExternalInput")
    # pkB [128, 160]: a8m (0:32, [ah, i, i'] step-masked), tri (32:160).
    # f32r so tri can feed the fp32r suffix matmul directly; the a8m part
    # is bitcast back to f32 for the exp.
    pkb = nc.dram_tensor("pkb", [128, 160], f32r, kind="ExternalInput")
    bwin = nc.dram_tensor("bwin", [128, NSTEP, 256], f32,
                          kind="ExternalInput")
    out = nc.dram_tensor("out", [NSTEP, D], f32, kind="ExternalOutput")

    with tile.TileContext(nc) as tc:
        with (
            tc.tile_pool(name="sb", bufs=1) as sb,
            tc.tile_pool(name="small", bufs=1) as small,
            tc.tile_pool(name="psA", bufs=1, space="PSUM") as psA,
            tc.tile_pool(name="psC", bufs=1, space="PSUM") as psC,
            tc.tile_pool(name="psD", bufs=1, space="PSUM") as psD,
        ):
            # ---- input DMAs, spread across queues ----
            pka_sb = sb.tile([128, 28], f32)
            nc.sync.dma_start(pka_sb[:], pka[:])
            pkb_sb = sb.tile([128, 160], f32r)
            nc.scalar.dma_start(pkb_sb[:], pkb[:])
            bwin_sb = sb.tile([128, NSTEP, 256], f32)
            for i in range(NSTEP):
                nc.gpsimd.dma_start(bwin_sb[:, i:i + 1, :], bwin[:, i:i + 1, :])

            # ---- exps (ACT, all from the exp_and_others table) ----
            epka = sb.tile([128, 16], f32r)
            nc.scalar.activation(epka[:], pka_sb[:, 0:16], Exp, scale=10.0)
            epkb = sb.tile([128, 32], f32r)
            nc.scalar.activation(epkb[:], pkb_sb[:, 0:32].bitcast(f32), Exp,
                                 scale=10.0)
            ewin = sb.tile([128, NSTEP, 256], f32r)
            for i in range(NSTEP):
                nc.scalar.activation(ewin[:, i:i + 1, :],
                                     bwin_sb[:, i:i + 1, :], Exp, scale=10.0)

            def ebv(bh):  # exp(b8)[:, bh, :] as f32
                return epka[:, 4 * bh:4 * bh + 4].bitcast(f32)

            def eav(ah):  # exp(a8)[:, ah, :] as f32
                return epka[:, 8 + 4 * ah:12 + 4 * ah].bitcast(f32)

            def eamv(ah, i):  # masked exp(a8) lhsT block, f32r
                o = 16 * ah + 4 * i
                return epkb[:, o:o + 4]

            ones4 = pka_sb[:, 16:20]
            mskZ = pka_sb[0:4, 20:24]
            mskT = pka_sb[0:4, 24:28]
            triv = pkb_sb[:, 32:160]

            # ---- suffix sums, both halves in one matmul ----
            # suf[p, (bh,i)] = sum_{q>p} EB_i[128*bh + q]
            suf_ps = psA.tile([128, 2, NSTEP], f32)
            nc.tensor.matmul(suf_ps[:].opt(), lhsT=triv, rhs=epka[:, 0:8],
                             start=True, stop=True)

            # ---- element products: U on DVE, V on GpSimd ----
            # scr: (u1, v1, u2, v2) so the fold can add halves [0:2]+[2:4].
            scr = sb.tile([128, 4, NSTEP], f32)
            nc.vector.tensor_tensor(out=scr[:, 0, :], in0=eav(0),
                                    in1=suf_ps[:, 1, :], op=mult)
            nc.vector.tensor_tensor(out=scr[:, 2, :], in0=eav(1),
                                    in1=suf_ps[:, 0, :], op=mult)
            nc.gpsimd.tensor_tensor(out=scr[:, 1, :], in0=eav(0),
                                    in1=ebv(1), op=mult)
            nc.gpsimd.tensor_tensor(out=scr[:, 3, :], in0=eav(1),
                                    in1=ebv(0), op=mult)

            # ---- partition reduction + broadcast to partitions 0-3 ----
            # red[p, 0, h, k, i]: (h, k) = (u1, v1 | u2, v2)
            # red[p, 1, k, h, i]: (k, h) = (zb0, zb1 | za0, za1)
            red_ps = psC.tile([NSTEP, 2, 2, 2, NSTEP], f32)
            nc.tensor.matmul(red_ps[:, 0, :, :, :].opt(), lhsT=ones4,
                             rhs=scr[:].opt(), start=True, stop=True)
            nc.tensor.matmul(red_ps[:, 1, :, :, :].opt(), lhsT=ones4,
                             rhs=epka[:, 0:16].bitcast(f32), start=True,
                             stop=True)

            # ---- folds (PSUM allows only one PSUM operand per op: copy
            # the 4x32 reduction block to SBUF once, fold from there) ----
            red_sb = small.tile([NSTEP, 2, 2, 2, NSTEP], f32)
            nc.vector.tensor_copy(out=red_sb[:], in_=red_ps[:])
            # sums1 = (U', V); U = U' + ZA1*ZB1 (cross-half correction).
            sums1 = small.tile([NSTEP, 2, NSTEP], f32)
            nc.vector.tensor_tensor(out=sums1[:], in0=red_sb[:, 0, 0, :, :],
                                    in1=red_sb[:, 0, 1, :, :], op=add)
            tzz = small.tile([NSTEP, NSTEP], f32)
            nc.gpsimd.tensor_tensor(out=tzz[:], in0=red_sb[:, 1, 0, 1, :],
                                    in1=red_sb[:, 1, 1, 1, :], op=mult)
            U = small.tile([NSTEP, NSTEP], f32)
            nc.vector.tensor_tensor(out=U[:], in0=sums1[:, 0, :], in1=tzz[:],
                                    op=add)
            # sums2 = (ZB, ZA); Z = ZB*ZA  (GpSimd, parallel with DVE)
            sums2 = small.tile([NSTEP, 2, NSTEP], f32)
            nc.gpsimd.tensor_tensor(out=sums2[:], in0=red_sb[:, 1, :, 0, :],
                                    in1=red_sb[:, 1, :, 1, :], op=add)
            Z = small.tile([NSTEP, NSTEP], f32)
            nc.gpsimd.tensor_tensor(out=Z[:], in0=sums2[:, 0, :],
                                    in1=sums2[:, 1, :], op=mult)

            zbi = small.tile([NSTEP, NSTEP], f32)
            nc.vector.reciprocal(zbi[:], Z[:])

            # ---- chain scale/bias prep ----
            # scale_i = 5 V_i zbi_i;  bias_i = (10U + 5V - 5Z)_i zbi_i
            V5 = small.tile([NSTEP, NSTEP], f32)
            nc.gpsimd.tensor_scalar(out=V5[:], in0=sums1[:, 1, :],
                                    scalar1=5.0, scalar2=None, op0=mult)
            Z5 = small.tile([NSTEP, NSTEP], f32)
            nc.gpsimd.tensor_scalar(out=Z5[:], in0=Z[:], scalar1=5.0,
                                    scalar2=None, op0=mult)
            pre5 = small.tile([NSTEP, NSTEP], f32)
            nc.gpsimd.tensor_tensor(out=pre5[:], in0=V5[:], in1=Z5[:],
                                    op=subtract)
            W2 = small.tile([NSTEP, NSTEP], f32)
            nc.vector.scalar_tensor_tensor(out=W2[:], in0=U[:], scalar=10.0,
                                           in1=pre5[:], op0=mult, op1=add)
            bias = small.tile([NSTEP, NSTEP], f32)
            nc.vector.tensor_tensor(out=bias[:], in0=W2[:], in1=zbi[:],
                                    op=mult)
            scale = small.tile([NSTEP, NSTEP], f32)
            nc.gpsimd.tensor_tensor(out=scale[:], in0=V5[:], in1=zbi[:],
                                    op=mult)

            # zsel[p] = zbi[p, p] via diag mask + pairwise adds (GpSimd).
            zm = small.tile([NSTEP, NSTEP], f32)
            nc.gpsimd.tensor_tensor(out=zm[:], in0=zbi[:], in1=mskZ, op=mult)
            z2 = small.tile([NSTEP, 2], f32)
            nc.gpsimd.tensor_tensor(out=z2[:], in0=zm[:, 0:2],
                                    in1=zm[:, 2:4], op=add)
            zsel = small.tile([NSTEP, 1], f32)
            nc.gpsimd.tensor_tensor(out=zsel[:], in0=z2[:, 0:1],
                                    in1=z2[:, 1:2], op=add)

            # ---- carry chain: 3 back-to-back Tanh ACTs ----
            T = small.tile([NSTEP, NSTEP], f32)
            nc.vector.memset(T[:, 0:1], T0)
            for i in range(NSTEP - 1):
                nc.scalar.activation(T[:, i + 1:i + 2], T[:, i:i + 1], Tanh,
                                     bias=bias[:, i:i + 1],
                                     scale=scale[:, i:i + 1])

            # ssel[p] = (1 + T[p, p]) / 2: mskT is 0.5*diag, so the masked
            # pairwise-add chain plus the 0.5 offset lands exactly there.
            tm = small.tile([NSTEP, NSTEP], f32)
            nc.gpsimd.tensor_tensor(out=tm[:], in0=T[:], in1=mskT, op=mult)
            t2 = small.tile([NSTEP, 2], f32)
            nc.gpsimd.tensor_tensor(out=t2[:], in0=tm[:, 0:2],
                                    in1=tm[:, 2:4], op=add)
            t3 = small.tile([NSTEP, 1], f32)
            nc.gpsimd.tensor_tensor(out=t3[:], in0=t2[:, 0:1],
                                    in1=t2[:, 1:2], op=add)
            ssel = small.tile([NSTEP, 1], f32)
            nc.gpsimd.tensor_scalar(out=ssel[:], in0=t3[:], scalar1=0.5,
                                    scalar2=None, op0=add)

            # ---- convolutions: 12 matmuls accumulate into prt[i, m] ----
            prt = psD.tile([NSTEP, 256], f32)
            for i in range(NSTEP):
                nc.tensor.matmul(prt[:], lhsT=eamv(0, i), rhs=ewin[:, i, :],
                                 start=(i == 0), stop=False)
                nc.tensor.matmul(prt[:, 0:128], lhsT=eamv(1, i),
                                 rhs=ewin[:, i, 128:256], start=False,
                                 stop=False)
                nc.tensor.matmul(prt[:, 128:256], lhsT=eamv(1, i),
                                 rhs=ewin[:, i, 0:128], start=False,
                                 stop=(i == NSTEP - 1))

            # ---- combine: out = zsel*cyc + ssel*(zsel*rot(cyc)-zsel*cyc)
            # pre = zsel*cyc and q = zsel*rot(cyc) each read PSUM once and
            # run before the chain finishes; only dz/comb trail the select.
            pre = sb.tile([NSTEP, 256], f32)
            nc.vector.tensor_scalar(out=pre[:], in0=prt[:],
                                    scalar1=zsel[:], scalar2=None, op0=mult)
            q = sb.tile([NSTEP, 255], f32)
            nc.vector.tensor_scalar(out=q[:], in0=prt[:, 0:255],
                                    scalar1=zsel[:], scalar2=None, op0=mult)
            dz = sb.tile([NSTEP, 256], f32)
            nc.vector.scalar_tensor_tensor(out=dz[:, 0:1],
                                           in0=prt[:, 255:256],
                                           scalar=zsel[:], in1=pre[:, 0:1],
                                           op0=mult, op1=subtract)
            nc.vector.tensor_tensor(out=dz[:, 1:256], in0=q[:],
                                    in1=pre[:, 1:256], op=subtract)
            comb = sb.tile([NSTEP, D], f32)
            nc.vector.scalar_tensor_tensor(out=comb[:], in0=dz[:],
                                           scalar=ssel[:], in1=pre[:],
                                           op0=mult, op1=add)
            nc.sync.dma_start(out[:], comb[:])

    nc.compile()
    return nc


def _structure_ok(W1, W2_sum, W2_carry):
    """Exact check that the tables are the canonical byte-add structure."""
    k = np.arange(NE)
    a = k >> 9
    b = (k >> 1) & 255
    c = k & 1
    total = a + b + c
    if W1.shape != (514, NE) or W2_sum.shape != (NE, D):
        return False
    if W2_carry.shape != (NE, 2):
        return False
    if not (W1[a, k] == 1.0).all():
        return False
    if not (W1[256 + b, k] == 1.0).all():
        return False
    if not (W1[512 + c, k] == 1.0).all():
        return False
    if np.abs(W1).sum(dtype=np.float64) != 3.0 * NE:
        return False
    if not (W2_sum[k, total & 255] == 1.0).all():
        return False
    if np.abs(W2_sum).sum(dtype=np.float64) != float(NE):
        return False
    if not (W2_carry[k, (total >= 256).astype(np.int64)] == 1.0).all():
        return False
    if np.abs(W2_carry).sum(dtype=np.float64) != float(NE):
        return False
    return True


def _numpy_fallback(a_emb, b_emb, W1, W2_sum, W2_carry):
    carry = np.zeros(2, dtype=np.float64)
    carry[0] = 1.0
    outs = []
    W1 = W1.astype(np.float64)
    for i in range(NSTEP):
        x = np.concatenate([a_emb[i], b_emb[i], carry]).astype(np.float64)
        scores = x @ W1
        z = (scores - 2.5) * 10.0
        z -= z.max()
        w = np.exp(z)
        w /= w.sum()
        outs.append(w @ W2_sum.astype(np.float64))
        carry = w @ W2_carry.astype(np.float64)
    return np.stack(outs).astype(np.float32)


def _prep_inputs(a_emb, b_emb):
    p = np.arange(128)
    # bwin[j, i, x] = b_emb[i, (j + x + 129) mod 256], one cyclic period
    b_ext = np.take(b_emb, (np.arange(383) + 129) % 256, axis=1)
    bwin = np.ascontiguousarray(
        np.lib.stride_tricks.sliding_window_view(b_ext, 256, axis=1)
        .transpose(1, 0, 2)
    ).astype(np.float32)
    # a8[p, ah, i] = a_emb[i, 128 ah + 127 - p]
    a_r = a_emb[:, ::-1]
    a8 = np.ascontiguousarray(
        a_r.reshape(NSTEP, 2, 128)[:, ::-1, :].transpose(2, 1, 0)
    ).astype(np.float32)
    # a8m: step-masked copy (off-step columns -5 -> exp(10x) ~ 2e-22)
    a8m = np.full((128, 2, NSTEP, NSTEP), -5.0, dtype=np.float32)
    for i in range(NSTEP):
        a8m[:, :, i, i] = a8[:, :, i]
    # b8[p, bh, i] = b_emb[i, 128 bh + p]
    b8 = np.ascontiguousarray(
        b_emb.reshape(NSTEP, 2, 128).transpose(2, 1, 0)
    ).astype(np.float32)
    ones4 = np.ones((128, 4), dtype=np.float32)
    mskZ = np.zeros((128, 4), dtype=np.float32)
    mskZ[np.arange(4), np.arange(4)] = 1.0
    # mskT = 0.5 * I so the ssel pairwise-add chain lands at
    # 0.5 + 0.5*t_diag = (1+t)/2 directly.
    mskT = 0.5 * mskZ
    pka = np.concatenate(
        [b8.reshape(128, 8), a8.reshape(128, 8), ones4, mskZ, mskT], axis=1)
    tri = (p[:, None] >= p[None, :] + 1).astype(np.float32)
    pkb = np.concatenate([a8m.reshape(128, 32), tri], axis=1)
    return {"pka": pka, "pkb": pkb, "bwin": bwin}


def kernel(a_emb, b_emb, W1, W2_sum, W2_carry):
    global LAST_EXEC_TIME_NS, LAST_RESULT
    a_emb = np.asarray(a_emb, dtype=np.float32)
    b_emb = np.asarray(b_emb, dtype=np.float32)
    W1 = np.asarray(W1, dtype=np.float32)
    W2_sum = np.asarray(W2_sum, dtype=np.float32)
    W2_carry = np.asarray(W2_carry, dtype=np.float32)

    if not _structure_ok(W1, W2_sum, W2_carry):
        return _numpy_fallback(a_emb, b_emb, W1, W2_sum, W2_carry)

    from concourse.bass_utils import run_bass_kernel_spmd

    if "nc" not in _CACHE:
        _CACHE["nc"] = _build_nc()
    nc = _CACHE["nc"]

    in_map = _prep_inputs(a_emb, b_emb)
    trace = os.environ.get("KERNEL_TRACE", "") == "1"
    res = run_bass_kernel_spmd(nc, [in_map], [0], trace=trace)
    LAST_EXEC_TIME_NS = res.exec_time_ns
    LAST_RESULT = res
    return np.asarray(res.results[0]["out"], dtype=np.float32)


# revision 25
# speedup vs baseline: 1.1019x; 1.0154x over previous
"""Trainium2 Bass kernel for nn_C4ByteTransformer (4-step carry-propagation
softmax table lookup).

Contract: kernel(**inputs) takes FULL inputs (a_emb[4,256], b_emb[4,256],
W1[514,131072], W2_sum[131072,256], W2_carry[131072,2]) and returns the full
[4,256] float32 output.

Math: the tables are the canonical byte-add lookup structure (verified
exactly on host, with a numpy fallback otherwise):
  scores_i[k] = a_emb[i, a] + b_emb[i, b] + carry[c],  k = 512a + 2b + c
  weights = softmax(10*(scores - 2.5));  out_i = weights @ W2_sum;
  carry' = weights @ W2_carry,  W2_sum[k, (a+b+c) & 255] = 1,
  W2_carry[k, a+b+c >= 256] = 1.
Because exp is multiplicative over the separable score, with
EA[a] = exp(10 a_emb[i,a]), EB[b] = exp(10 b_emb[i,b]) and
s = sigmoid(20 carry_1 - 10) (= F1/(F0+F1)):
  out_i[m] = ((1-s) cyc[m] + s cyc[(m-1) mod 256]) / (ZA ZB)
  carry'_1 = (U + V s) / (ZA ZB)
where cyc = 256-point cyclic convolution of EA and EB,
U = sum_{a+b>=256} EA[a]EB[b], V = sum_{a+b=255} EA[a]EB[b].
The 131072-entry table never has to be touched.

V2 latency structure (the kernel is launch-overhead dominated; ~13.1us of
the exec time is fixed preamble/DMA-latency/teardown measured with a
trivial kernel):
 - The carry recursion is rewritten in tanh form:
     t_{i+1} = tanh(5 vz_i t_i + (10 uz_i + 5 vz_i - 5)),  s = (1+t)/2,
   which the ACT engine evaluates as ONE Tanh per step with per-partition
   scale/bias APs. Tanh lives in the same activation-function set as Exp
   (exp_and_others), so no table swap and no DVE round-trips: the whole
   chain is 3 back-to-back ACT ops.
 - U/V/Z sums: one [128x128] triangular matmul (tri rides the pk DMA as
   f32, bitcast to f32r) gives within-half suffix sums for both halves at
   once; element products and folds are split across DVE and GpSimd; one
   ones-lhsT matmul pair reduces partitions and broadcasts to partitions
   0-3.
 - Final combine: out = zsel*(cyc + ssel*(rot(cyc)-cyc)). d = rot-cyc and
   the zsel pre-scales run on DVE while the chain finishes, leaving a
   single [4,256] op after the step-select.
 - DMA: pkA (small, gates everything) on the sync queue, pkB (a8m+tri) on
   the tensor queue, the four Hankel windows split over gpsimd+vector
   queues. Constants (ones, diag mask) ride pkA; one activation-table
   load; 5 input DMA instructions total.
"""

import os

import numpy as np

NSTEP = 4
D = 256
NE = 131072

_CACHE = {}

LAST_EXEC_TIME_NS = None
LAST_RESULT = None

T0 = float(np.tanh(-5.0))  # chain state for step 0 (s0 = sigmoid(-10))


def _build_nc():
    import concourse.bacc as bacc
    import concourse.mybir as mybir
    import concourse.tile as tile

    f32 = mybir.dt.float32
    f32r = mybir.dt.float32r
    mult = mybir.AluOpType.mult
    add = mybir.AluOpType.add
    subtract = mybir.AluOpType.subtract
    Exp = mybir.ActivationFunctionType.Exp
    Tanh = mybir.ActivationFunctionType.Tanh

    nc = bacc.Bacc("TRN2", target_bir_lowering=False, debug=False,
                   num_devices=1)

    # pkA [128, 28]: b8 (0:8, [bh,i]), a8 (8:16, [ah,i]), ones4 (16:20),
    # mskZ = diag(1.0) rows 0-3 (20:24), mskT = diag(0.5) (24:28).
    pka = nc.dram_tensor("pka", [128, 28], f32, kind="ExternalInput")
    # pkB [128, 160]: a8m (0:32, [ah, i, i'] step-masked), tri (32:160).
    # f32r so tri can feed the fp32r suffix matmul directly; the a8m part
    # is bitcast back to f32 for the exp.
    pkb = nc.dram_tensor("pkb", [128, 160], f32r, kind="ExternalInput")
    bwin = nc.dram_tensor("bwin", [128, NSTEP, 256], f32,
                          kind="ExternalInput")
    out = nc.dram_tensor("out", [NSTEP, D], f32, kind="ExternalOutput")

    with tile.TileContext(nc) as tc:
        with (
            tc.tile_pool(name="sb", bufs=1) as sb,
            tc.tile_pool(name="small", bufs=1) as small,
            tc.tile_pool(name="psA", bufs=1, space="PSUM") as psA,
            tc.tile_pool(name="psC", bufs=1, space="PSUM") as psC,
            tc.tile_pool(name="psD", bufs=1, space="PSUM") as psD,
        ):
            # ---- input DMAs, spread across queues ----
            pka_sb = sb.tile([128, 28], f32)
            nc.sync.dma_start(pka_sb[:], pka[:])
            pkb_sb = sb.tile([128, 160], f32r)
            nc.scalar.dma_start(pkb_sb[:], pkb[:])
            bwin_sb = sb.tile([128, NSTEP, 256], f32)
            for i in range(NSTEP):
                nc.gpsimd.dma_start(bwin_sb[:, i:i + 1, :], bwin[:, i:i + 1, :])

            # ---- exps (ACT, all from the exp_and_others table) ----
            epka = sb.tile([128, 16], f32r)
            nc.scalar.activation(epka[:], pka_sb[:, 0:16], Exp, scale=10.0)
            epkb = sb.tile([128, 32], f32r)
            nc.scalar.activation(epkb[:], pkb_sb[:, 0:32].bitcast(f32), Exp,
                                 scale=10.0)
            ewin = sb.tile([128, NSTEP, 256], f32r)
            for i in range(NSTEP):
                nc.scalar.activation(ewin[:, i:i + 1, :],
                                     bwin_sb[:, i:i + 1, :], Exp, scale=10.0)

            def ebv(bh):  # exp(b8)[:, bh, :] as f32
                return epka[:, 4 * bh:4 * bh + 4].bitcast(f32)

            def eav(ah):  # exp(a8)[:, ah, :] as f32
                return epka[:, 8 + 4 * ah:12 + 4 * ah].bitcast(f32)

            def eamv(ah, i):  # masked exp(a8) lhsT block, f32r
                o = 16 * ah + 4 * i
                return epkb[:, o:o + 4]

            ones4 = pka_sb[:, 16:20]
            mskZ = pka_sb[0:4, 20:24]
            mskT = pka_sb[0:4, 24:28]
            triv = pkb_sb[:, 32:160]

            # ---- suffix sums, both halves in one matmul ----
            # suf[p, (bh,i)] = sum_{q>p} EB_i[128*bh + q]
            suf_ps = psA.tile([128, 2, NSTEP], f32)
            nc.tensor.matmul(suf_ps[:].opt(), lhsT=triv, rhs=epka[:, 0:8],
                             start=True, stop=True)

            # ---- element products (all DVE; v-products first, they only
            # need epka while the u-products wait on the suffix matmul) ----
            # scr: (u1, v1, u2, v2) so the fold can add halves [0:2]+[2:4].
            scr = sb.tile([128, 4, NSTEP], f32)
            nc.vector.tensor_tensor(out=scr[:, 1, :], in0=eav(0),
                                    in1=ebv(1), op=mult)
            nc.vector.tensor_tensor(out=scr[:, 3, :], in0=eav(1),
                                    in1=ebv(0), op=mult)
            nc.vector.tensor_tensor(out=scr[:, 0, :], in0=eav(0),
                                    in1=suf_ps[:, 1, :], op=mult)
            nc.vector.tensor_tensor(out=scr[:, 2, :], in0=eav(1),
                                    in1=suf_ps[:, 0, :], op=mult)

            # ---- partition reduction + broadcast to partitions 0-3 ----
            # red[p, 0, h, k, i]: (h, k) = (u1, v1 | u2, v2)
            # red[p, 1, k, h, i]: (k, h) = (zb0, zb1 | za0, za1)
            red_ps = psC.tile([NSTEP, 2, 2, 2, NSTEP], f32)
            nc.tensor.matmul(red_ps[:, 0, :, :, :].opt(), lhsT=ones4,
                             rhs=scr[:].opt(), start=True, stop=True)
            nc.tensor.matmul(red_ps[:, 1, :, :, :].opt(), lhsT=ones4,
                             rhs=epka[:, 0:16].bitcast(f32), start=True,
                             stop=True)

            # ---- folds (PSUM allows only one PSUM operand per op: copy
            # the 4x32 reduction block to SBUF once, fold from there) ----
            red_sb = small.tile([NSTEP, 2, 2, 2, NSTEP], f32)
            nc.vector.tensor_copy(out=red_sb[:], in_=red_ps[:])
            # sums1 = (U', V); U = U' + ZA1*ZB1 (cross-half correction).
            sums1 = small.tile([NSTEP, 2, NSTEP], f32)
            nc.vector.tensor_tensor(out=sums1[:], in0=red_sb[:, 0, 0, :, :],
                                    in1=red_sb[:, 0, 1, :, :], op=add)
            V5 = small.tile([NSTEP, NSTEP], f32)
            nc.vector.tensor_scalar(out=V5[:], in0=sums1[:, 1, :],
                                    scalar1=5.0, scalar2=None, op0=mult)
            tzz = small.tile([NSTEP, NSTEP], f32)
            nc.gpsimd.tensor_tensor(out=tzz[:], in0=red_sb[:, 1, 0, 1, :],
                                    in1=red_sb[:, 1, 1, 1, :], op=mult)
            U = small.tile([NSTEP, NSTEP], f32)
            nc.vector.tensor_tensor(out=U[:], in0=sums1[:, 0, :], in1=tzz[:],
                                    op=add)
            # W1 = 10U + 5V (DVE) while GpSimd folds Z = (ZB0+ZB1)(ZA0+ZA1)
            W1 = small.tile([NSTEP, NSTEP], f32)
            nc.vector.scalar_tensor_tensor(out=W1[:], in0=U[:], scalar=10.0,
                                           in1=V5[:], op0=mult, op1=add)
            sums2 = small.tile([NSTEP, 2, NSTEP], f32)
            nc.gpsimd.tensor_tensor(out=sums2[:], in0=red_sb[:, 1, :, 0, :],
                                    in1=red_sb[:, 1, :, 1, :], op=add)
            Z = small.tile([NSTEP, NSTEP], f32)
            nc.gpsimd.tensor_tensor(out=Z[:], in0=sums2[:, 0, :],
                                    in1=sums2[:, 1, :], op=mult)

            zbi = small.tile([NSTEP, NSTEP], f32)
            nc.vector.reciprocal(zbi[:], Z[:])

            # ---- chain scale/bias ----
            # scale_i = 5 V_i zbi_i;  bias_i = (10U + 5V)_i zbi_i - 5
            bW = small.tile([NSTEP, NSTEP], f32)
            nc.vector.tensor_tensor(out=bW[:], in0=W1[:], in1=zbi[:],
                                    op=mult)
            bias = small.tile([NSTEP, NSTEP], f32)
            nc.vector.tensor_scalar(out=bias[:], in0=bW[:], scalar1=-5.0,
                                    scalar2=None, op0=add)
            scale = small.tile([NSTEP, NSTEP], f32)
            nc.vector.tensor_tensor(out=scale[:], in0=V5[:], in1=zbi[:],
                                    op=mult)

            # zsel[p] = zbi[p, p]: fused mask-mult + free-reduce in one STT.
            zm = small.tile([NSTEP, NSTEP], f32)
            zsel = small.tile([NSTEP, 1], f32)
            nc.vector.scalar_tensor_tensor(out=zm[:], in0=zbi[:], scalar=1.0,
                                           in1=mskZ, op0=mult, op1=mult,
                                           accum_out=zsel[:])

            # ---- carry chain: 3 back-to-back Tanh ACTs ----
            T = small.tile([NSTEP, NSTEP], f32)
            nc.vector.memset(T[:, 0:1], T0)
            for i in range(NSTEP - 1):
                nc.scalar.activation(T[:, i + 1:i + 2], T[:, i:i + 1], Tanh,
                                     bias=bias[:, i:i + 1],
                                     scale=scale[:, i:i + 1])

            # ---- convolutions: 12 matmuls accumulate into prt[i, m] ----
            prt = psD.tile([NSTEP, 256], f32)
            for i in range(NSTEP):
                nc.tensor.matmul(prt[:], lhsT=eamv(0, i), rhs=ewin[:, i, :],
                                 start=(i == 0), stop=False)
                nc.tensor.matmul(prt[:, 0:128], lhsT=eamv(1, i),
                                 rhs=ewin[:, i, 128:256], start=False,
                                 stop=False)
                nc.tensor.matmul(prt[:, 128:256], lhsT=eamv(1, i),
                                 rhs=ewin[:, i, 0:128], start=False,
                                 stop=(i == NSTEP - 1))

            # ---- combine: out = zsel*cyc + ssel*(zsel*rot(cyc)-zsel*cyc)
            # pre = zsel*cyc and q = zsel*rot(cyc) each read PSUM once and
            # run before the chain finishes; only dz/comb trail the select.
            pre = sb.tile([NSTEP, 256], f32)
            nc.vector.tensor_scalar(out=pre[:], in0=prt[:],
                                    scalar1=zsel[:], scalar2=None, op0=mult)
            q = sb.tile([NSTEP, 255], f32)
            nc.vector.tensor_scalar(out=q[:], in0=prt[:, 0:255],
                                    scalar1=zsel[:], scalar2=None, op0=mult)
            dz = sb.tile([NSTEP, 256], f32)
            nc.vector.scalar_tensor_tensor(out=dz[:, 0:1],
                                           in0=prt[:, 255:256],
                                           scalar=zsel[:], in1=pre[:, 0:1],
                                           op0=mult, op1=subtract)
            nc.vector.tensor_tensor(out=dz[:, 1:256], in0=q[:],
                                    in1=pre[:, 1:256], op=subtract)
            # ssel[p] = (1 + T[p, p]) / 2: one fused (0.5*T)*diag with
            # free-dim accumulate, then +0.5. Emitted after the big
            # pre/q/dz ops so the in-order DVE queue never stalls on the
            # chain before issuing them.
            tm = small.tile([NSTEP, NSTEP], f32)
            thalf = small.tile([NSTEP, 1], f32)
            nc.vector.scalar_tensor_tensor(out=tm[:], in0=T[:], scalar=0.5,
                                           in1=mskZ, op0=mult, op1=mult,
                                           accum_out=thalf[:])
            ssel = small.tile([NSTEP, 1], f32)
            nc.vector.tensor_scalar(out=ssel[:], in0=thalf[:], scalar1=0.5,
                                    scalar2=None, op0=add)
            comb = sb.tile([NSTEP, D], f32)
            nc.vector.scalar_tensor_tensor(out=comb[:], in0=dz[:],
                                           scalar=ssel[:], in1=pre[:],
                                           op0=mult, op1=add)
            nc.sync.dma_start(out[:], comb[:])

    nc.compile()
    return nc


def _structure_ok(W1, W2_sum, W2_carry):
    """Exact check that the tables are the canonical byte-add structure."""
    k = np.arange(NE)
    a = k >> 9
    b = (k >> 1) & 255
    c = k & 1
    total = a + b + c
    if W1.shape != (514, NE) or W2_sum.shape != (NE, D):
        return False
    if W2_carry.shape != (NE, 2):
        return False
    if not (W1[a, k] == 1.0).all():
        return False
    if not (W1[256 + b, k] == 1.0).all():
        return False
    if not (W1[512 + c, k] == 1.0).all():
        return False
    if np.abs(W1).sum(dtype=np.float64) != 3.0 * NE:
        return False
    if not (W2_sum[k, total & 255] == 1.0).all():
        return False
    if np.abs(W2_sum).sum(dtype=np.float64) != float(NE):
        return False
    if not (W2_carry[k, (total >= 256).astype(np.int64)] == 1.0).all():
        return False
    if np.abs(W2_carry).sum(dtype=np.float64) != float(NE):
        return False
    return True


def _numpy_fallback(a_emb, b_emb, W1, W2_sum, W2_carry):
    carry = np.zeros(2, dtype=np.float64)
    carry[0] = 1.0
    outs = []
    W1 = W1.astype(np.float64)
    for i in range(NSTEP):
        x = np.concatenate([a_emb[i], b_emb[i], carry]).astype(np.float64)
        scores = x @ W1
        z = (scores - 2.5) * 10.0
        z -= z.max()
        w = np.exp(z)
        w /= w.sum()
        outs.append(w @ W2_sum.astype(np.float64))
        carry = w @ W2_carry.astype(np.float64)
    return np.stack(outs).astype(np.float32)


def _prep_inputs(a_emb, b_emb):
    p = np.arange(128)
    # bwin[j, i, x] = b_emb[i, (j + x + 129) mod 256], one cyclic period
    b_ext = np.take(b_emb, (np.arange(383) + 129) % 256, axis=1)
    bwin = np.ascontiguousarray(
        np.lib.stride_tricks.sliding_window_view(b_ext, 256, axis=1)
        .transpose(1, 0, 2)
    ).astype(np.float32)
    # a8[p, ah, i] = a_emb[i, 128 ah + 127 - p]
    a_r = a_emb[:, ::-1]
    a8 = np.ascontiguousarray(
        a_r.reshape(NSTEP, 2, 128)[:, ::-1, :].transpose(2, 1, 0)
    ).astype(np.float32)
    # a8m: step-masked copy (off-step columns -5 -> exp(10x) ~ 2e-22)
    a8m = np.full((128, 2, NSTEP, NSTEP), -5.0, dtype=np.float32)
    for i in range(NSTEP):
        a8m[:, :, i, i] = a8[:, :, i]
    # b8[p, bh, i] = b_emb[i, 128 bh + p]
    b8 = np.ascontiguousarray(
        b_emb.reshape(NSTEP, 2, 128).transpose(2, 1, 0)
    ).astype(np.float32)
    ones4 = np.ones((128, 4), dtype=np.float32)
    mskZ = np.zeros((128, 4), dtype=np.float32)
    mskZ[np.arange(4), np.arange(4)] = 1.0
    # mskT = 0.5 * I so the ssel pairwise-add chain lands at
    # 0.5 + 0.5*t_diag = (1+t)/2 directly.
    mskT = 0.5 * mskZ
    pka = np.concatenate(
        [b8.reshape(128, 8), a8.reshape(128, 8), ones4, mskZ, mskT], axis=1)
    tri = (p[:, None] >= p[None, :] + 1).astype(np.float32)
    pkb = np.concatenate([a8m.reshape(128, 32), tri], axis=1)
    return {"pka": pka, "pkb": pkb, "bwin": bwin}


def kernel(a_emb, b_emb, W1, W2_sum, W2_carry):
    global LAST_EXEC_TIME_NS, LAST_RESULT
    a_emb = np.asarray(a_emb, dtype=np.float32)
    b_emb = np.asarray(b_emb, dtype=np.float32)
    W1 = np.asarray(W1, dtype=np.float32)
    W2_sum = np.asarray(W2_sum, dtype=np.float32)
    W2_carry = np.asarray(W2_carry, dtype=np.float32)

    if not _structure_ok(W1, W2_sum, W2_carry):
        return _numpy_fallback(a_emb, b_emb, W1, W2_sum, W2_carry)

    from concourse.bass_utils import run_bass_kernel_spmd

    if "nc" not in _CACHE:
        _CACHE["nc"] = _build_nc()
    nc = _CACHE["nc"]

    in_map = _prep_inputs(a_emb, b_emb)
    trace = os.environ.get("KERNEL_TRACE", "") == "1"
    res = run_bass_kernel_spmd(nc, [in_map], [0], trace=trace)
    LAST_EXEC_TIME_NS = res.exec_time_ns
    LAST_RESULT = res
    return np.asarray(res.results[0]["out"], dtype=np.float32)


# revision 30
# speedup vs baseline: 1.1759x; 1.0671x over previous
"""Trainium2 Bass kernel for nn_C4ByteTransformer (4-step carry-propagation
softmax table lookup).

Contract: kernel(**inputs) takes FULL inputs (a_emb[4,256], b_emb[4,256],
W1[514,131072], W2_sum[131072,256], W2_carry[131072,2]) and returns the full
[4,256] float32 output.

Math: the tables are the canonical byte-add lookup structure (verified
exactly on host, with a numpy fallback otherwise):
  scores_i[k] = a_emb[i, a] + b_emb[i, b] + carry[c],  k = 512a + 2b + c
  weights = softmax(10*(scores - 2.5));  out_i = weights @ W2_sum;
  carry' = weights @ W2_carry,  W2_sum[k, (a+b+c) & 255] = 1,
  W2_carry[k, a+b+c >= 256] = 1.
Because exp is multiplicative over the separable score, with
EA[a] = exp(10 a_emb[i,a]), EB[b] = exp(10 b_emb[i,b]) and
s = sigmoid(20 carry_1 - 10) (= F1/(F0+F1)):
  out_i[m] = ((1-s) cyc[m] + s cyc[(m-1) mod 256]) / (ZA ZB)
  carry'_1 = (U + V s) / (ZA ZB)
where cyc = 256-point cyclic convolution of EA and EB,
U = sum_{a+b>=256} EA[a]EB[b], V = sum_{a+b=255} EA[a]EB[b].
The 131072-entry table never has to be touched.

V2 latency structure (the kernel is launch-overhead dominated; ~13.1us of
the exec time is fixed preamble/DMA-latency/teardown measured with a
trivial kernel):
 - The carry recursion is rewritten in tanh form:
     t_{i+1} = tanh(5 vz_i t_i + (10 uz_i + 5 vz_i - 5)),  s = (1+t)/2,
   which the ACT engine evaluates as ONE Tanh per step with per-partition
   scale/bias APs. Tanh lives in the same activation-function set as Exp
   (exp_and_others), so no table swap and no DVE round-trips: the whole
   chain is 3 back-to-back ACT ops.
 - U/V/Z sums: one [128x128] triangular matmul (tri rides the pk DMA as
   f32, bitcast to f32r) gives within-half suffix sums for both halves at
   once; element products and folds are split across DVE and GpSimd; one
   ones-lhsT matmul pair reduces partitions and broadcasts to partitions
   0-3.
 - Final combine: out = zsel*(cyc + ssel*(rot(cyc)-cyc)). d = rot-cyc and
   the zsel pre-scales run on DVE while the chain finishes, leaving a
   single [4,256] op after the step-select.
 - DMA: pkA (small, gates everything) on the sync queue, pkB (a8m+tri) on
   the tensor queue, the four Hankel windows split over gpsimd+vector
   queues. Constants (ones, diag mask) ride pkA; one activation-table
   load; 5 input DMA instructions total.
"""

import os

import numpy as np

NSTEP = 4
D = 256
NE = 131072

_CACHE = {}

LAST_EXEC_TIME_NS = None
LAST_RESULT = None

T0 = float(np.tanh(-5.0))  # chain state for step 0 (s0 = sigmoid(-10))


def _build_nc():
    import concourse.bacc as bacc
    import concourse.mybir as mybir
    import concourse.tile as tile

    f32 = mybir.dt.float32
    f32r = mybir.dt.float32r
    f16 = mybir.dt.float16
    mult = mybir.AluOpType.mult
    add = mybir.AluOpType.add
    subtract = mybir.AluOpType.subtract
    Exp = mybir.ActivationFunctionType.Exp
    Tanh = mybir.ActivationFunctionType.Tanh

    nc = bacc.Bacc("TRN2", target_bir_lowering=False, debug=False,
                   num_devices=1)

    # pkA [128, 28]: b8 (0:8, [bh,i]), a8 (8:16, [ah,i]), ones4 (16:20),
    # mskZ = diag(1.0) rows 0-3 (20:24), mskT = diag(0.5) (24:28).
    pka = nc.dram_tensor("pka", [128, 28], f32, kind="ExternalInput")
    # pkB [128, 160]: a8m (0:32, [ah, i, i'] step-masked), tri (32:160).
    # f32r so tri can feed the fp32r suffix matmul directly; the a8m part
    # is bitcast back to f32 for the exp.
    pkb = nc.dram_tensor("pkb", [128, 160], f32r, kind="ExternalInput")
    # fp16 halves the window DMA packet time; exp(10b) quantization error
    # from fp16 b-values is ~2.4e-3 worst-case, well inside tolerance.
    bwin = nc.dram_tensor("bwin", [128, NSTEP, 256], f16,
                          kind="ExternalInput")
    out = nc.dram_tensor("out", [NSTEP, D], f32, kind="ExternalOutput")

    with tile.TileContext(nc) as tc:
        with (
            tc.tile_pool(name="sb", bufs=1) as sb,
            tc.tile_pool(name="small", bufs=1) as small,
            tc.tile_pool(name="psA", bufs=1, space="PSUM") as psA,
            tc.tile_pool(name="psC", bufs=1, space="PSUM") as psC,
            tc.tile_pool(name="psD", bufs=1, space="PSUM") as psD,
        ):
            # ---- input DMAs, spread across queues ----
            pka_sb = sb.tile([128, 28], f32)
            nc.sync.dma_start(pka_sb[:], pka[:])
            pkb_sb = sb.tile([128, 160], f32r)
            nc.scalar.dma_start(pkb_sb[:], pkb[:])
            bwin_sb = sb.tile([128, NSTEP, 256], f16)
            for i in range(NSTEP):
                nc.gpsimd.dma_start(bwin_sb[:, i:i + 1, :], bwin[:, i:i + 1, :])

            # ---- exps (ACT, all from the exp_and_others table) ----
            epka = sb.tile([128, 16], f32r)
            nc.scalar.activation(epka[:], pka_sb[:, 0:16], Exp, scale=10.0)
            epkb = sb.tile([128, 32], f32r)
            nc.scalar.activation(epkb[:], pkb_sb[:, 0:32].bitcast(f32), Exp,
                                 scale=10.0)
            ewin = sb.tile([128, NSTEP, 256], f32r)
            for i in range(NSTEP):
                nc.scalar.activation(ewin[:, i:i + 1, :],
                                     bwin_sb[:, i:i + 1, :], Exp, scale=10.0)

            def ebv(bh):  # exp(b8)[:, bh, :] as f32
                return epka[:, 4 * bh:4 * bh + 4].bitcast(f32)

            def eav(ah):  # exp(a8)[:, ah, :] as f32
                return epka[:, 8 + 4 * ah:12 + 4 * ah].bitcast(f32)

            def eamv(ah, i):  # masked exp(a8) lhsT block, f32r
                o = 16 * ah + 4 * i
                return epkb[:, o:o + 4]

            ones4 = pka_sb[:, 16:20]
            mskZ = pka_sb[0:4, 20:24]
            mskT = pka_sb[0:4, 24:28]
            triv = pkb_sb[:, 32:160]

            # ---- suffix sums, both halves in one matmul ----
            # suf[p, (bh,i)] = sum_{q>p} EB_i[128*bh + q]
            suf_ps = psA.tile([128, 2, NSTEP], f32)
            nc.tensor.matmul(suf_ps[:].opt(), lhsT=triv, rhs=epka[:, 0:8],
                             start=True, stop=True)

            # ---- element products (all DVE; v-products first, they only
            # need epka while the u-products wait on the suffix matmul) ----
            # scr: (u1, v1, u2, v2) so the fold can add halves [0:2]+[2:4].
            scr = sb.tile([128, 4, NSTEP], f32)
            nc.vector.tensor_tensor(out=scr[:, 1, :], in0=eav(0),
                                    in1=ebv(1), op=mult)
            nc.vector.tensor_tensor(out=scr[:, 3, :], in0=eav(1),
                                    in1=ebv(0), op=mult)
            nc.vector.tensor_tensor(out=scr[:, 0, :], in0=eav(0),
                                    in1=suf_ps[:, 1, :], op=mult)
            nc.vector.tensor_tensor(out=scr[:, 2, :], in0=eav(1),
                                    in1=suf_ps[:, 0, :], op=mult)

            # ---- partition reduction + broadcast to partitions 0-3 ----
            # red[p, 0, h, k, i]: (h, k) = (u1, v1 | u2, v2)
            # red[p, 1, k, h, i]: (k, h) = (zb0, zb1 | za0, za1)
            red_ps = psC.tile([NSTEP, 2, 2, 2, NSTEP], f32)
            nc.tensor.matmul(red_ps[:, 0, :, :, :].opt(), lhsT=ones4,
                             rhs=scr[:].opt(), start=True, stop=True)
            nc.tensor.matmul(red_ps[:, 1, :, :, :].opt(), lhsT=ones4,
                             rhs=epka[:, 0:16].bitcast(f32), start=True,
                             stop=True)

            # ---- folds (PSUM allows only one PSUM operand per op: copy
            # the 4x32 reduction block to SBUF once, fold from there) ----
            red_sb = small.tile([NSTEP, 2, 2, 2, NSTEP], f32)
            nc.vector.tensor_copy(out=red_sb[:], in_=red_ps[:])
            # sums1 = (U', V); U = U' + ZA1*ZB1 (cross-half correction).
            sums1 = small.tile([NSTEP, 2, NSTEP], f32)
            nc.vector.tensor_tensor(out=sums1[:], in0=red_sb[:, 0, 0, :, :],
                                    in1=red_sb[:, 0, 1, :, :], op=add)
            V5 = small.tile([NSTEP, NSTEP], f32)
            nc.vector.tensor_scalar(out=V5[:], in0=sums1[:, 1, :],
                                    scalar1=5.0, scalar2=None, op0=mult)
            tzz = small.tile([NSTEP, NSTEP], f32)
            nc.gpsimd.tensor_tensor(out=tzz[:], in0=red_sb[:, 1, 0, 1, :],
                                    in1=red_sb[:, 1, 1, 1, :], op=mult)
            U = small.tile([NSTEP, NSTEP], f32)
            nc.vector.tensor_tensor(out=U[:], in0=sums1[:, 0, :], in1=tzz[:],
                                    op=add)
            # W1 = 10U + 5V (DVE) while GpSimd folds Z = (ZB0+ZB1)(ZA0+ZA1)
            W1 = small.tile([NSTEP, NSTEP], f32)
            nc.vector.scalar_tensor_tensor(out=W1[:], in0=U[:], scalar=10.0,
                                           in1=V5[:], op0=mult, op1=add)
            sums2 = small.tile([NSTEP, 2, NSTEP], f32)
            nc.gpsimd.tensor_tensor(out=sums2[:], in0=red_sb[:, 1, :, 0, :],
                                    in1=red_sb[:, 1, :, 1, :], op=add)
            Z = small.tile([NSTEP, NSTEP], f32)
            nc.gpsimd.tensor_tensor(out=Z[:], in0=sums2[:, 0, :],
                                    in1=sums2[:, 1, :], op=mult)

            zbi = small.tile([NSTEP, NSTEP], f32)
            nc.vector.reciprocal(zbi[:], Z[:])

            # ---- chain scale/bias ----
            # scale_i = 5 V_i zbi_i;  bias_i = (10U + 5V)_i zbi_i - 5
            bW = small.tile([NSTEP, NSTEP], f32)
            nc.vector.tensor_tensor(out=bW[:], in0=W1[:], in1=zbi[:],
                                    op=mult)
            bias = small.tile([NSTEP, NSTEP], f32)
            nc.vector.tensor_scalar(out=bias[:], in0=bW[:], scalar1=-5.0,
                                    scalar2=None, op0=add)
            scale = small.tile([NSTEP, NSTEP], f32)
            nc.vector.tensor_tensor(out=scale[:], in0=V5[:], in1=zbi[:],
                                    op=mult)

            # zsel[p] = zbi[p, p]: fused mask-mult + free-reduce in one STT.
            zm = small.tile([NSTEP, NSTEP], f32)
            zsel = small.tile([NSTEP, 1], f32)
            nc.vector.scalar_tensor_tensor(out=zm[:], in0=zbi[:], scalar=1.0,
                                           in1=mskZ, op0=mult, op1=mult,
                                           accum_out=zsel[:])

            # ---- carry chain: 3 back-to-back Tanh ACTs ----
            T = small.tile([NSTEP, NSTEP], f32)
            nc.vector.memset(T[:, 0:1], T0)
            for i in range(NSTEP - 1):
                nc.scalar.activation(T[:, i + 1:i + 2], T[:, i:i + 1], Tanh,
                                     bias=bias[:, i:i + 1],
                                     scale=scale[:, i:i + 1])

            # ---- convolutions: 12 matmuls accumulate into prt[i, m] ----
            prt = psD.tile([NSTEP, 256], f32)
            for i in range(NSTEP):
                nc.tensor.matmul(prt[:], lhsT=eamv(0, i), rhs=ewin[:, i, :],
                                 start=(i == 0), stop=False)
                nc.tensor.matmul(prt[:, 0:128], lhsT=eamv(1, i),
                                 rhs=ewin[:, i, 128:256], start=False,
                                 stop=False)
                nc.tensor.matmul(prt[:, 128:256], lhsT=eamv(1, i),
                                 rhs=ewin[:, i, 0:128], start=False,
                                 stop=(i == NSTEP - 1))

            # ---- combine: out = zsel*cyc + ssel*(zsel*rot(cyc)-zsel*cyc)
            # pre = zsel*cyc and q = zsel*rot(cyc) each read PSUM once and
            # run before the chain finishes; only dz/comb trail the select.
            pre = sb.tile([NSTEP, 256], f32)
            nc.vector.tensor_scalar(out=pre[:], in0=prt[:],
                                    scalar1=zsel[:], scalar2=None, op0=mult)
            q = sb.tile([NSTEP, 255], f32)
            nc.vector.tensor_scalar(out=q[:], in0=prt[:, 0:255],
                                    scalar1=zsel[:], scalar2=None, op0=mult)
            dz = sb.tile([NSTEP, 256], f32)
            nc.vector.scalar_tensor_tensor(out=dz[:, 0:1],
                                           in0=prt[:, 255:256],
                                           scalar=zsel[:], in1=pre[:, 0:1],
                                           op0=mult, op1=subtract)
            dz_tt = nc.vector.tensor_tensor(out=dz[:, 1:256], in0=q[:],
                                            in1=pre[:, 1:256], op=subtract)
            # ssel[p] = (1 + T[p, p]) / 2: one fused (0.5*T)*diag with
            # free-dim accumulate, then +0.5. Pinned after dz so the
            # scheduler can't hoist the chain-gated select ahead of the
            # big pre/q/dz ops on the DVE queue.
            tm = small.tile([NSTEP, NSTEP], f32)
            thalf = small.tile([NSTEP, 1], f32)
            ssel_stt = nc.vector.scalar_tensor_tensor(
                out=tm[:], in0=T[:], scalar=0.5, in1=mskZ, op0=mult,
                op1=mult, accum_out=thalf[:])
            tile.add_dep_helper(ssel_stt.ins, dz_tt.ins, False,
                                "big combine ops before chain select")
            ssel = small.tile([NSTEP, 1], f32)
            nc.vector.tensor_scalar(out=ssel[:], in0=thalf[:], scalar1=0.5,
                                    scalar2=None, op0=add)
            comb = sb.tile([NSTEP, D], f32)
            nc.vector.scalar_tensor_tensor(out=comb[:], in0=dz[:],
                                           scalar=ssel[:], in1=pre[:],
                                           op0=mult, op1=add)
            nc.sync.dma_start(out[:], comb[:])

    nc.compile()
    return nc


def _structure_ok(W1, W2_sum, W2_carry):
    """Exact check that the tables are the canonical byte-add structure."""
    k = np.arange(NE)
    a = k >> 9
    b = (k >> 1) & 255
    c = k & 1
    total = a + b + c
    if W1.shape != (514, NE) or W2_sum.shape != (NE, D):
        return False
    if W2_carry.shape != (NE, 2):
        return False
    if not (W1[a, k] == 1.0).all():
        return False
    if not (W1[256 + b, k] == 1.0).all():
        return False
    if not (W1[512 + c, k] == 1.0).all():
        return False
    if np.abs(W1).sum(dtype=np.float64) != 3.0 * NE:
        return False
    if not (W2_sum[k, total & 255] == 1.0).all():
        return False
    if np.abs(W2_sum).sum(dtype=np.float64) != float(NE):
        return False
    if not (W2_carry[k, (total >= 256).astype(np.int64)] == 1.0).all():
        return False
    if np.abs(W2_carry).sum(dtype=np.float64) != float(NE):
        return False
    return True


def _numpy_fallback(a_emb, b_emb, W1, W2_sum, W2_carry):
    carry = np.zeros(2, dtype=np.float64)
    carry[0] = 1.0
    outs = []
    W1 = W1.astype(np.float64)
    for i in range(NSTEP):
        x = np.concatenate([a_emb[i], b_emb[i], carry]).astype(np.float64)
        scores = x @ W1
        z = (scores - 2.5) * 10.0
        z -= z.max()
        w = np.exp(z)
        w /= w.sum()
        outs.append(w @ W2_sum.astype(np.float64))
        carry = w @ W2_carry.astype(np.float64)
    return np.stack(outs).astype(np.float32)


def _prep_inputs(a_emb, b_emb):
    p = np.arange(128)
    # bwin[j, i, x] = b_emb[i, (j + x + 129) mod 256], one cyclic period
    b_ext = np.take(b_emb, (np.arange(383) + 129) % 256, axis=1)
    bwin = np.ascontiguousarray(
        np.lib.stride_tricks.sliding_window_view(b_ext, 256, axis=1)
        .transpose(1, 0, 2)
    ).astype(np.float16)
    # a8[p, ah, i] = a_emb[i, 128 ah + 127 - p]
    a_r = a_emb[:, ::-1]
    a8 = np.ascontiguousarray(
        a_r.reshape(NSTEP, 2, 128)[:, ::-1, :].transpose(2, 1, 0)
    ).astype(np.float32)
    # a8m: step-masked copy (off-step columns -5 -> exp(10x) ~ 2e-22)
    a8m = np.full((128, 2, NSTEP, NSTEP), -5.0, dtype=np.float32)
    for i in range(NSTEP):
        a8m[:, :, i, i] = a8[:, :, i]
    # b8[p, bh, i] = b_emb[i, 128 bh + p]
    b8 = np.ascontiguousarray(
        b_emb.reshape(NSTEP, 2, 128).transpose(2, 1, 0)
    ).astype(np.float32)
    ones4 = np.ones((128, 4), dtype=np.float32)
    mskZ = np.zeros((128, 4), dtype=np.float32)
    mskZ[np.arange(4), np.arange(4)] = 1.0
    # mskT = 0.5 * I so the ssel pairwise-add chain lands at
    # 0.5 + 0.5*t_diag = (1+t)/2 directly.
    mskT = 0.5 * mskZ
    pka = np.concatenate(
        [b8.reshape(128, 8), a8.reshape(128, 8), ones4, mskZ, mskT], axis=1)
    tri = (p[:, None] >= p[None, :] + 1).astype(np.float32)
    pkb = np.concatenate([a8m.reshape(128, 32), tri], axis=1)
    return {"pka": pka, "pkb": pkb, "bwin": bwin}


def kernel(a_emb, b_emb, W1, W2_sum, W2_carry):
    global LAST_EXEC_TIME_NS, LAST_RESULT
    a_emb = np.asarray(a_emb, dtype=np.float32)
    b_emb = np.asarray(b_emb, dtype=np.float32)
    W1 = np.asarray(W1, dtype=np.float32)
    W2_sum = np.asarray(W2_sum, dtype=np.float32)
    W2_carry = np.asarray(W2_carry, dtype=np.float32)

    if not _structure_ok(W1, W2_sum, W2_carry):
        return _numpy_fallback(a_emb, b_emb, W1, W2_sum, W2_carry)

    from concourse.bass_utils import run_bass_kernel_spmd

    if "nc" not in _CACHE:
        _CACHE["nc"] = _build_nc()
    nc = _CACHE["nc"]

    in_map = _prep_inputs(a_emb, b_emb)
    trace = os.environ.get("KERNEL_TRACE", "") == "1"
    res = run_bass_kernel_spmd(nc, [in_map], [0], trace=trace)
    LAST_EXEC_TIME_NS = res.exec_time_ns
    LAST_RESULT = res
    return np.asarray(res.results[0]["out"], dtype=np.float32)


# revision 38
# speedup vs baseline: 1.1934x; 1.0149x over previous
"""Trainium2 Bass kernel for nn_C4ByteTransformer (4-step carry-propagation
softmax table lookup).

Contract: kernel(**inputs) takes FULL inputs (a_emb[4,256], b_emb[4,256],
W1[514,131072], W2_sum[131072,256], W2_carry[131072,2]) and returns the full
[4,256] float32 output.

Math: the tables are the canonical byte-add lookup structure (verified
exactly on host, with a numpy fallback otherwise):
  scores_i[k] = a_emb[i, a] + b_emb[i, b] + carry[c],  k = 512a + 2b + c
  weights = softmax(10*(scores - 2.5));  out_i = weights @ W2_sum;
  carry' = weights @ W2_carry,  W2_sum[k, (a+b+c) & 255] = 1,
  W2_carry[k, a+b+c >= 256] = 1.
Because exp is multiplicative over the separable score, with
EA[a] = exp(10 a_emb[i,a]), EB[b] = exp(10 b_emb[i,b]) and
s = sigmoid(20 carry_1 - 10) (= F1/(F0+F1)):
  out_i[m] = ((1-s) cyc[m] + s cyc[(m-1) mod 256]) / (ZA ZB)
  carry'_1 = (U + V s) / (ZA ZB)
where cyc = 256-point cyclic convolution of EA and EB,
U = sum_{a+b>=256} EA[a]EB[b], V = sum_{a+b=255} EA[a]EB[b].
The 131072-entry table never has to be touched.

V2 latency structure (the kernel is launch-overhead dominated; ~13.1us of
the exec time is fixed preamble/DMA-latency/teardown measured with a
trivial kernel):
 - The carry recursion is rewritten in tanh form:
     t_{i+1} = tanh(5 vz_i t_i + (10 uz_i + 5 vz_i - 5)),  s = (1+t)/2,
   which the ACT engine evaluates as ONE Tanh per step with per-partition
   scale/bias APs. Tanh lives in the same activation-function set as Exp
   (exp_and_others), so no table swap and no DVE round-trips: the whole
   chain is 3 back-to-back ACT ops.
 - U/V/Z sums: one [128x128] triangular matmul (tri rides the pk DMA as
   f32, bitcast to f32r) gives within-half suffix sums for both halves at
   once; element products and folds are split across DVE and GpSimd; one
   ones-lhsT matmul pair reduces partitions and broadcasts to partitions
   0-3.
 - Final combine: out = zsel*(cyc + ssel*(rot(cyc)-cyc)). d = rot-cyc and
   the zsel pre-scales run on DVE while the chain finishes, leaving a
   single [4,256] op after the step-select.
 - DMA: pkA (small, gates everything) on the sync queue, pkB (a8m+tri) on
   the tensor queue, the four Hankel windows split over gpsimd+vector
   queues. Constants (ones, diag mask) ride pkA; one activation-table
   load; 5 input DMA instructions total.
"""

import os

import numpy as np

NSTEP = 4
D = 256
NE = 131072

_CACHE = {}

LAST_EXEC_TIME_NS = None
LAST_RESULT = None

T0 = float(np.tanh(-5.0))  # chain state for step 0 (s0 = sigmoid(-10))


def _build_nc():
    import concourse.bacc as bacc
    import concourse.mybir as mybir
    import concourse.tile as tile

    f32 = mybir.dt.float32
    f32r = mybir.dt.float32r
    f16 = mybir.dt.float16
    mult = mybir.AluOpType.mult
    add = mybir.AluOpType.add
    subtract = mybir.AluOpType.subtract
    Exp = mybir.ActivationFunctionType.Exp
    Tanh = mybir.ActivationFunctionType.Tanh

    nc = bacc.Bacc("TRN2", target_bir_lowering=False, debug=False,
                   num_devices=1)

    # pkA [128, 28]: b8 (0:8, [bh,i]), a8 (8:16, [ah,i]), ones4 (16:20),
    # mskZ = diag(1.0) rows 0-3 (20:24), mskT = diag(0.5) (24:28).
    pka = nc.dram_tensor("pka", [128, 28], f32, kind="ExternalInput")
    # pkB [128, 160]: a8m (0:32, [ah, i, i'] step-masked), tri (32:160).
    # f32r so tri can feed the fp32r suffix matmul directly; the a8m part
    # is bitcast back to f32 for the exp.
    pkb = nc.dram_tensor("pkb", [128, 160], f32r, kind="ExternalInput")
    # fp16 halves the window DMA packet time; exp(10b) quantization error
    # from fp16 b-values is ~2.4e-3 worst-case, well inside tolerance.
    bwin = nc.dram_tensor("bwin", [128, NSTEP, 256], f16,
                          kind="ExternalInput")
    # fp16 output: DVE 2-byte ops run in 2x mode and the values are
    # normalized (~[0,1]) before quantization; host casts back to f32.
    out = nc.dram_tensor("out", [NSTEP, D], f16, kind="ExternalOutput")

    with tile.TileContext(nc) as tc:
        with (
            tc.tile_pool(name="sb", bufs=1) as sb,
            tc.tile_pool(name="small", bufs=1) as small,
            tc.tile_pool(name="psA", bufs=1, space="PSUM") as psA,
            tc.tile_pool(name="psC", bufs=1, space="PSUM") as psC,
            tc.tile_pool(name="psD", bufs=1, space="PSUM") as psD,
        ):
            # ---- input DMAs, spread across queues ----
            pka_sb = sb.tile([128, 28], f32)
            nc.sync.dma_start(pka_sb[:], pka[:])
            pkb_sb = sb.tile([128, 160], f32r)
            nc.scalar.dma_start(pkb_sb[:], pkb[:])
            bwin_sb = sb.tile([128, NSTEP, 256], f16)
            # windows 0/1 on the gpsimd queue, 2/3 on sync behind pkA:
            # two queues generate descriptors in parallel, so the last
            # window lands ~0.7us earlier than four serial descriptors.
            nc.gpsimd.dma_start(bwin_sb[:, 0:1, :], bwin[:, 0:1, :])
            nc.sync.dma_start(bwin_sb[:, 2:3, :], bwin[:, 2:3, :])
            nc.gpsimd.dma_start(bwin_sb[:, 1:2, :], bwin[:, 1:2, :])
            nc.sync.dma_start(bwin_sb[:, 3:4, :], bwin[:, 3:4, :])

            # ---- exps (ACT, all from the exp_and_others table) ----
            epka = sb.tile([128, 16], f32r)
            nc.scalar.activation(epka[:], pka_sb[:, 0:16], Exp, scale=10.0)
            # fp16 conv operands: PE runs 16-bit matmuls at 1 cycle/row
            # (f32r needs 4 at these sizes); exp(10*a|b) <= e^10 fits fp16,
            # and the -5-masked lanes underflow to exactly 0.
            epkb = sb.tile([128, 32], f16)
            nc.scalar.activation(epkb[:], pkb_sb[:, 0:32].bitcast(f32), Exp,
                                 scale=10.0)
            ewin = sb.tile([128, NSTEP, 256], f16)
            for i in range(NSTEP):
                nc.scalar.activation(ewin[:, i:i + 1, :],
                                     bwin_sb[:, i:i + 1, :], Exp, scale=10.0)

            def ebv(bh):  # exp(b8)[:, bh, :] as f32
                return epka[:, 4 * bh:4 * bh + 4].bitcast(f32)

            def eav(ah):  # exp(a8)[:, ah, :] as f32
                return epka[:, 8 + 4 * ah:12 + 4 * ah].bitcast(f32)

            def eamv(ah, i):  # masked exp(a8) lhsT block, f16
                o = 16 * ah + 4 * i
                return epkb[:, o:o + 4]

            ones4 = pka_sb[:, 16:20]
            mskZ = pka_sb[0:4, 20:24]
            mskT = pka_sb[0:4, 24:28]
            triv = pkb_sb[:, 32:160]

            # ---- suffix sums, both halves in one matmul ----
            # suf[p, (bh,i)] = sum_{q>p} EB_i[128*bh + q]
            suf_ps = psA.tile([128, 2, NSTEP], f32)
            nc.tensor.matmul(suf_ps[:].opt(), lhsT=triv, rhs=epka[:, 0:8],
                             start=True, stop=True)

            # ---- element products (all DVE; v-products first, they only
            # need epka while the u-products wait on the suffix matmul) ----
            # scr: (u1, v1, u2, v2) so the fold can add halves [0:2]+[2:4].
            scr = sb.tile([128, 4, NSTEP], f32)
            nc.vector.tensor_tensor(out=scr[:, 1, :], in0=eav(0),
                                    in1=ebv(1), op=mult)
            nc.vector.tensor_tensor(out=scr[:, 3, :], in0=eav(1),
                                    in1=ebv(0), op=mult)
            nc.vector.tensor_tensor(out=scr[:, 0, :], in0=eav(0),
                                    in1=suf_ps[:, 1, :], op=mult)
            nc.vector.tensor_tensor(out=scr[:, 2, :], in0=eav(1),
                                    in1=suf_ps[:, 0, :], op=mult)

            # ---- partition reduction + broadcast to partitions 0-3 ----
            # red[p, 0, h, k, i]: (h, k) = (u1, v1 | u2, v2)
            # red[p, 1, k, h, i]: (k, h) = (zb0, zb1 | za0, za1)
            red_ps = psC.tile([NSTEP, 2, 2, 2, NSTEP], f32)
            nc.tensor.matmul(red_ps[:, 0, :, :, :].opt(), lhsT=ones4,
                             rhs=scr[:].opt(), start=True, stop=True)
            nc.tensor.matmul(red_ps[:, 1, :, :, :].opt(), lhsT=ones4,
                             rhs=epka[:, 0:16].bitcast(f32), start=True,
                             stop=True)

            # ---- folds (PSUM allows only one PSUM operand per op: copy
            # the 4x32 reduction block to SBUF once, fold from there) ----
            red_sb = small.tile([NSTEP, 2, 2, 2, NSTEP], f32)
            nc.vector.tensor_copy(out=red_sb[:], in_=red_ps[:])
            # sums1 = (U', V); U = U' + ZA1*ZB1 (cross-half correction).
            sums1 = small.tile([NSTEP, 2, NSTEP], f32)
            nc.vector.tensor_tensor(out=sums1[:], in0=red_sb[:, 0, 0, :, :],
                                    in1=red_sb[:, 0, 1, :, :], op=add)
            V5 = small.tile([NSTEP, NSTEP], f32)
            nc.vector.tensor_scalar(out=V5[:], in0=sums1[:, 1, :],
                                    scalar1=5.0, scalar2=None, op0=mult)
            tzz = small.tile([NSTEP, NSTEP], f32)
            nc.gpsimd.tensor_tensor(out=tzz[:], in0=red_sb[:, 1, 0, 1, :],
                                    in1=red_sb[:, 1, 1, 1, :], op=mult)
            U = small.tile([NSTEP, NSTEP], f32)
            nc.vector.tensor_tensor(out=U[:], in0=sums1[:, 0, :], in1=tzz[:],
                                    op=add)
            # W1 = 10U + 5V (DVE) while GpSimd folds Z = (ZB0+ZB1)(ZA0+ZA1)
            W1 = small.tile([NSTEP, NSTEP], f32)
            nc.vector.scalar_tensor_tensor(out=W1[:], in0=U[:], scalar=10.0,
                                           in1=V5[:], op0=mult, op1=add)
            sums2 = small.tile([NSTEP, 2, NSTEP], f32)
            nc.gpsimd.tensor_tensor(out=sums2[:], in0=red_sb[:, 1, :, 0, :],
                                    in1=red_sb[:, 1, :, 1, :], op=add)
            Z = small.tile([NSTEP, NSTEP], f32)
            nc.gpsimd.tensor_tensor(out=Z[:], in0=sums2[:, 0, :],
                                    in1=sums2[:, 1, :], op=mult)

            zbi = small.tile([NSTEP, NSTEP], f32)
            nc.vector.reciprocal(zbi[:], Z[:])

            # ---- chain scale/bias ----
            # scale_i = 5 V_i zbi_i;  bias_i = (10U + 5V)_i zbi_i - 5
            bW = small.tile([NSTEP, NSTEP], f32)
            nc.vector.tensor_tensor(out=bW[:], in0=W1[:], in1=zbi[:],
                                    op=mult)
            scale = small.tile([NSTEP, NSTEP], f32)
            nc.vector.tensor_tensor(out=scale[:], in0=V5[:], in1=zbi[:],
                                    op=mult)
            bias = small.tile([NSTEP, NSTEP], f32)
            bias_ts = nc.vector.tensor_scalar(out=bias[:], in0=bW[:],
                                              scalar1=-5.0, scalar2=None,
                                              op0=add)

            # zsel[p] = zbi[p, p]: fused mask-mult + free-reduce in one STT.
            # Pinned after bias so the scheduler can't slot it into the
            # recip->bias stretch of the chain-critical DVE queue.
            zm = small.tile([NSTEP, NSTEP], f32)
            zsel = small.tile([NSTEP, 1], f32)
            zsel_stt = nc.vector.scalar_tensor_tensor(
                out=zm[:], in0=zbi[:], scalar=1.0, in1=mskZ, op0=mult,
                op1=mult, accum_out=zsel[:])
            tile.add_dep_helper(zsel_stt.ins, bias_ts.ins, False,
                                "chain bias before zsel")

            # ---- carry chain: 3 back-to-back Tanh ACTs ----
            T = small.tile([NSTEP, NSTEP], f32)
            nc.vector.memset(T[:, 0:1], T0)
            for i in range(NSTEP - 1):
                nc.scalar.activation(T[:, i + 1:i + 2], T[:, i:i + 1], Tanh,
                                     bias=bias[:, i:i + 1],
                                     scale=scale[:, i:i + 1])

            # ---- convolutions: 12 matmuls accumulate into prt[i, m] ----
            prt = psD.tile([NSTEP, 256], f32)
            for i in range(NSTEP):
                nc.tensor.matmul(prt[:], lhsT=eamv(0, i), rhs=ewin[:, i, :],
                                 start=(i == 0), stop=False)
                nc.tensor.matmul(prt[:, 0:128], lhsT=eamv(1, i),
                                 rhs=ewin[:, i, 128:256], start=False,
                                 stop=False)
                nc.tensor.matmul(prt[:, 128:256], lhsT=eamv(1, i),
                                 rhs=ewin[:, i, 0:128], start=False,
                                 stop=(i == NSTEP - 1))

            # ---- combine: out = zsel*cyc + ssel*(zsel*rot(cyc)-zsel*cyc)
            # pre = zsel*cyc and q = zsel*rot(cyc) each read PSUM once and
            # run before the chain finishes; only dz/comb trail the select.
            # fp16 pre/q/dz/comb: values are normalized by zsel (~[0,1]),
            # and all-2-byte SBUF operands put the DVE in its 2x mode for
            # dz and comb.
            pre = sb.tile([NSTEP, 256], f16)
            nc.vector.tensor_scalar(out=pre[:], in0=prt[:],
                                    scalar1=zsel[:], scalar2=None, op0=mult)
            q = sb.tile([NSTEP, 255], f16)
            nc.vector.tensor_scalar(out=q[:], in0=prt[:, 0:255],
                                    scalar1=zsel[:], scalar2=None, op0=mult)
            dz = sb.tile([NSTEP, 256], f16)
            nc.vector.scalar_tensor_tensor(out=dz[:, 0:1],
                                           in0=prt[:, 255:256],
                                           scalar=zsel[:], in1=pre[:, 0:1],
                                           op0=mult, op1=subtract)
            dz_tt = nc.vector.tensor_tensor(out=dz[:, 1:256], in0=q[:],
                                            in1=pre[:, 1:256], op=subtract)
            # ssel[p] = (1 + T[p, p]) / 2: one fused (0.5*T)*diag with
            # free-dim accumulate, then +0.5. Pinned after dz so the
            # scheduler can't hoist the chain-gated select ahead of the
            # big pre/q/dz ops on the DVE queue.
            tm = small.tile([NSTEP, NSTEP], f32)
            thalf = small.tile([NSTEP, 1], f32)
            ssel_stt = nc.vector.scalar_tensor_tensor(
                out=tm[:], in0=T[:], scalar=0.5, in1=mskZ, op0=mult,
                op1=mult, accum_out=thalf[:])
            tile.add_dep_helper(ssel_stt.ins, dz_tt.ins, False,
                                "big combine ops before chain select")
            ssel = small.tile([NSTEP, 1], f32)
            nc.vector.tensor_scalar(out=ssel[:], in0=thalf[:], scalar1=0.5,
                                    scalar2=None, op0=add)
            comb = sb.tile([NSTEP, D], f16)
            nc.vector.scalar_tensor_tensor(out=comb[:], in0=dz[:],
                                           scalar=ssel[:], in1=pre[:],
                                           op0=mult, op1=add)
            nc.sync.dma_start(out[:], comb[:])

    nc.compile()
    return nc


def _structure_ok(W1, W2_sum, W2_carry):
    """Exact check that the tables are the canonical byte-add structure."""
    k = np.arange(NE)
    a = k >> 9
    b = (k >> 1) & 255
    c = k & 1
    total = a + b + c
    if W1.shape != (514, NE) or W2_sum.shape != (NE, D):
        return False
    if W2_carry.shape != (NE, 2):
        return False
    if not (W1[a, k] == 1.0).all():
        return False
    if not (W1[256 + b, k] == 1.0).all():
        return False
    if not (W1[512 + c, k] == 1.0).all():
        return False
    if np.abs(W1).sum(dtype=np.float64) != 3.0 * NE:
        return False
    if not (W2_sum[k, total & 255] == 1.0).all():
        return False
    if np.abs(W2_sum).sum(dtype=np.float64) != float(NE):
        return False
    if not (W2_carry[k, (total >= 256).astype(np.int64)] == 1.0).all():
        return False
    if np.abs(W2_carry).sum(dtype=np.float64) != float(NE):
        return False
    return True


def _numpy_fallback(a_emb, b_emb, W1, W2_sum, W2_carry):
    carry = np.zeros(2, dtype=np.float64)
    carry[0] = 1.0
    outs = []
    W1 = W1.astype(np.float64)
    for i in range(NSTEP):
        x = np.concatenate([a_emb[i], b_emb[i], carry]).astype(np.float64)
        scores = x @ W1
        z = (scores - 2.5) * 10.0
        z -= z.max()
        w = np.exp(z)
        w /= w.sum()
        outs.append(w @ W2_sum.astype(np.float64))
        carry = w @ W2_carry.astype(np.float64)
    return np.stack(outs).astype(np.float32)


def _prep_inputs(a_emb, b_emb):
    p = np.arange(128)
    # bwin[j, i, x] = b_emb[i, (j + x + 129) mod 256], one cyclic period
    b_ext = np.take(b_emb, (np.arange(383) + 129) % 256, axis=1)
    bwin = np.ascontiguousarray(
        np.lib.stride_tricks.sliding_window_view(b_ext, 256, axis=1)
        .transpose(1, 0, 2)
    ).astype(np.float16)
    # a8[p, ah, i] = a_emb[i, 128 ah + 127 - p]
    a_r = a_emb[:, ::-1]
    a8 = np.ascontiguousarray(
        a_r.reshape(NSTEP, 2, 128)[:, ::-1, :].transpose(2, 1, 0)
    ).astype(np.float32)
    # a8m: step-masked copy (off-step columns -5 -> exp(10x) ~ 2e-22)
    a8m = np.full((128, 2, NSTEP, NSTEP), -5.0, dtype=np.float32)
    for i in range(NSTEP):
        a8m[:, :, i, i] = a8[:, :, i]
    # b8[p, bh, i] = b_emb[i, 128 bh + p]
    b8 = np.ascontiguousarray(
        b_emb.reshape(NSTEP, 2, 128).transpose(2, 1, 0)
    ).astype(np.float32)
    ones4 = np.ones((128, 4), dtype=np.float32)
    mskZ = np.zeros((128, 4), dtype=np.float32)
    mskZ[np.arange(4), np.arange(4)] = 1.0
    # mskT = 0.5 * I so the ssel pairwise-add chain lands at
    # 0.5 + 0.5*t_diag = (1+t)/2 directly.
    mskT = 0.5 * mskZ
    pka = np.concatenate(
        [b8.reshape(128, 8), a8.reshape(128, 8), ones4, mskZ, mskT], axis=1)
    tri = (p[:, None] >= p[None, :] + 1).astype(np.float32)
    pkb = np.concatenate([a8m.reshape(128, 32), tri], axis=1)
    return {"pka": pka, "pkb": pkb, "bwin": bwin}


def kernel(a_emb, b_emb, W1, W2_sum, W2_carry):
    global LAST_EXEC_TIME_NS, LAST_RESULT
    a_emb = np.asarray(a_emb, dtype=np.float32)
    b_emb = np.asarray(b_emb, dtype=np.float32)
    W1 = np.asarray(W1, dtype=np.float32)
    W2_sum = np.asarray(W2_sum, dtype=np.float32)
    W2_carry = np.asarray(W2_carry, dtype=np.float32)

    if not _structure_ok(W1, W2_sum, W2_carry):
        return _numpy_fallback(a_emb, b_emb, W1, W2_sum, W2_carry)

    from concourse.bass_utils import run_bass_kernel_spmd

    if "nc" not in _CACHE:
        _CACHE["nc"] = _build_nc()
    nc = _CACHE["nc"]

    in_map = _prep_inputs(a_emb, b_emb)
    trace = os.environ.get("KERNEL_TRACE", "") == "1"
    res = run_bass_kernel_spmd(nc, [in_map], [0], trace=trace)
    LAST_EXEC_TIME_NS = res.exec_time_ns
    LAST_RESULT = res
    return np.asarray(res.results[0]["out"]).astype(np.float32)
